# revision 1
# baseline (speedup 1.0000x reference)
"""Trainium2 Bass kernel for filtered backprojection (FBP).

reference semantics:
    filtered = irfft(rfft(sinos, axis=-1) * kernel, n=512, axis=-1)
    out[b,i,j] = sum_phi lerp(filtered[b,phi,:], u(phi,i,j)) * DPHI
with u affine in (i,j) per angle.

Device pipeline (8 NeuronCores, SPMD, no collectives):
  F0  h = irfft(kernel) via small matmuls against a host irfft matrix
  F1  circulant C[s,t] = h[(t-s)%512] built via per-partition indirect DMA
  F2  filter (replicated on every core): filtered rows = sinoT.T @ C
      (bf16 matmuls) over all 720 angles, chunked 512 cols at a time,
      written to local DRAM as filt[phi, t, b] bf16.
  B   backprojection: image in 16x16 tiles; per (angle,tile) only a 36-wide
      detector window contributes. D4 symmetry (8 exact pixel-grid
      symmetries) dedups weight blocks 8x. Per canonical tile: the 8
      member tiles' windows are gathered into two sign-grouped tiles
      (4 slots each); matmuls use 32-wide lhsT strips via tile_position
      (32-col LDWEIGHTS pipelines for free on this toolchain; 128-col
      does not). Final 4-way sigma-permuted merge happens on host.

Weights/idx tables are pure geometry -> precomputed on host in fp64.
"""
import numpy as np
import ml_dtypes

# ---------------- geometry constants ----------------
PHI, T, H, W = 720, 512, 256, 256
RHO = float(np.sqrt(2.0))
DPHI = float(np.pi) / PHI
DT = 2.0 * RHO / T
T0 = -RHO + 0.5 * DT
DX = 2.0 / H
TS, NT = 16, 16            # tile size / tiles per side
KWIN = 36                  # weight k-window (taps per (class,tile))
GWIN = 36                  # gathered k-window (36*32 bf16 = 2304B, 256B-aligned)
NCLS = 180                 # D4 angle classes
NG = 8                     # group size
NCORE = 8
B = 32
NU_PER_CORE = 16           # canonical-tile units per core

# ---------------- D4 group tables ----------------
def _mats():
    out = []
    for swap in (False, True):
        for sx in (1, -1):
            for sy in (1, -1):
                if not swap:
                    out.append(np.array([[sx, 0], [0, sy]]))
                else:
                    out.append(np.array([[0, sx], [sy, 0]]))
    return out

MATS = _mats()

def _angle_dirs():
    th = (np.arange(PHI) + 0.5) * DPHI
    return np.stack([np.cos(th), np.sin(th)], axis=1)

def _angle_maps():
    dirs = _angle_dirs()
    amap = np.zeros((NG, PHI), np.int64)
    smap = np.zeros((NG, PHI), np.int64)
    for gi, M in enumerate(MATS):
        v = dirs @ M          # row a: M^T d(a)
        dots = v @ dirs.T     # [a, a']
        ip = np.argmax(dots, axis=1)
        im = np.argmin(dots, axis=1)
        for a in range(PHI):
            if dots[a, ip[a]] > 1 - 1e-9:
                amap[gi, a], smap[gi, a] = ip[a], 1
            elif dots[a, im[a]] < -1 + 1e-9:
                amap[gi, a], smap[gi, a] = im[a], -1
            else:
                raise AssertionError((gi, a))
    inv_a = np.zeros_like(amap); inv_s = np.zeros_like(smap)
    for gi in range(NG):
        inv_a[gi, amap[gi]] = np.arange(PHI)
        inv_s[gi, amap[gi]] = smap[gi]
    return amap, smap, inv_a, inv_s

def _pixel_map(M):
    def f(i, j):
        def comp(row, i, j):
            if row[0] == 1:  return i
            if row[0] == -1: return 255 - i
            if row[1] == 1:  return j
            return 255 - j
        return comp(M[0], i, j), comp(M[1], i, j)
    return f

def _tile_pixel_maps():
    gT, gP = [], []
    for M in MATS:
        f = _pixel_map(M)
        tm = np.zeros(NT * NT, np.int64)
        pm = np.zeros(TS * TS, np.int64)
        for ti in range(NT):
            for tj in range(NT):
                a1 = f(ti * TS, tj * TS)
                a2 = f(ti * TS + TS - 1, tj * TS + TS - 1)
                tm[ti * NT + tj] = (min(a1[0], a2[0]) // TS) * NT + min(a1[1], a2[1]) // TS
        for pi in range(TS):
            for pj in range(TS):
                i_, j_ = f(pi, pj)
                pm[pi * TS + pj] = (i_ % TS) * TS + (j_ % TS)
        gT.append(tm); gP.append(pm)
    return gT, gP

def _uABC(a):
    th = (np.asarray(a, np.float64) + 0.5) * DPHI
    A = DX * np.cos(th) / DT
    Bc = DX * np.sin(th) / DT
    C = ((-1 + 0.5 * DX) * (np.cos(th) + np.sin(th)) - T0) / DT
    return A, Bc, C

_plan_cache = None

def build_plan():
    global _plan_cache
    if _plan_cache is not None:
        return _plan_cache
    amap, smap, inv_a, inv_s = _angle_maps()
    gT, gP = _tile_pixel_maps()
    rho_i = next(gi for gi, M in enumerate(MATS)
                 if M[0][0] == -1 and M[0][1] == 0 and M[1][1] == -1)
    assert all(amap[rho_i, a] == a and smap[rho_i, a] == -1 for a in range(PHI))

    # per-g uniform sign + band of the inverse angle map over classes 0..179
    bands = np.zeros(NG, np.int64)
    signs = np.zeros(NG, np.int64)
    for gi in range(NG):
        aa = inv_a[gi, :NCLS]
        ss = inv_s[gi, :NCLS]
        assert (ss == ss[0]).all(), gi
        bd = aa // NCLS
        assert (bd == bd[0]).all(), (gi, np.unique(bd))
        bands[gi] = bd[0]
        signs[gi] = ss[0]

    # tile orbits under D4; canonical taus: tau < rho180(tau)
    rho_t = gT[rho_i]
    tseen = np.zeros(NT * NT, bool)
    orbits = []
    for t in range(NT * NT):
        if tseen[t]: continue
        tiles = []
        for gi in range(NG):
            tt = gT[gi][t]
            if not tseen[tt]:
                tseen[tt] = True
                tiles.append(tt)
        orbits.append(tiles)
    canon = [[t for t in tiles if t < rho_t[t]] for tiles in orbits]
    big = [i for i, o in enumerate(orbits) if len(o) == 8]
    small = [i for i, o in enumerate(orbits) if len(o) == 4]
    assert len(big) == 28 and len(small) == 8, (len(big), len(small))
    core_nus = []   # per core: list of 16 canonical tile ids
    for c in range(7):
        nus = []
        for oi in big[c * 4:(c + 1) * 4]:
            nus.extend(canon[oi])
        assert len(nus) == 16
        core_nus.append(nus)
    nus7 = []
    for oi in small:
        nus7.extend(canon[oi])
    assert len(nus7) == 16
    core_nus.append(nus7)

    # per (class, tile): fp64 window base (4-aligned), weights
    ii = np.arange(TS, dtype=np.float64)
    def u_tile(a, tile):
        ti, tj = divmod(int(tile), NT)
        A, Bc, C = _uABC(a)
        return (A * (ti * TS + ii[:, None]) + Bc * (tj * TS + ii[None, :]) + C)

    def base4_of(a, tile):
        u = u_tile(a, tile)
        b = int(np.floor(u.min()))
        b4 = (b // 4) * 4
        return int(np.clip(b4, 0, 476))

    # W tables per core: WA [16,128,KWIN,256], WB [16,52,KWIN,256] bf16
    # gather idx per core: [16, 8, 128, 12] int16 (wrapped, replicated)
    # merge spec per core: [16, 8] -> (tile m, sigma gi)  (m from gT)
    kk = np.arange(KWIN, dtype=np.float64)
    WAs, WBs, GIs, merges = [], [], [], []
    for c in range(NCORE):
        Wfull = np.zeros((NU_PER_CORE, NCLS, KWIN, TS * TS), np.float64)
        gidx = np.full((NU_PER_CORE, NG, 192), -1, np.int64)
        mspec = np.zeros((NU_PER_CORE, NG, 2), np.int64)
        for j, tau in enumerate(core_nus[c]):
            U = np.stack([u_tile(cc, tau).reshape(-1) for cc in range(NCLS)])  # [180,256]
            b4 = np.array([base4_of(cc, tau) for cc in range(NCLS)])           # [180]
            rel = U[:, None, :] - b4[:, None, None] - kk[None, :, None]
            Wk = np.maximum(0.0, 1.0 - np.abs(rel))
            tap = b4[:, None] + kk[None, :]
            Wk[(tap < 0) | (tap >= T), :] = 0.0
            Wfull[j] = Wk * DPHI
            for gi in range(NG):
                s = signs[gi]
                aa = inv_a[gi, :NCLS]          # member angles [180]
                arel = aa % NCLS
                mb = b4 if s == 1 else 476 - b4
                idxv = (arel * T + mb) // 4
                assert (idxv * 4 == arel * T + mb).all()
                assert idxv.max() < 32768
                gidx[j, gi, :NCLS] = idxv
                mspec[j, gi] = (gT[gi][tau], gi)
        WAs.append(Wfull[:, :128].astype(ml_dtypes.bfloat16))
        WBs.append(Wfull[:, 128:].astype(ml_dtypes.bfloat16))
        # fused per-(nu, band) gather rows: [P-member chunk0 (128),
        # M-member chunk0 (128), chunk1 pair (P at partitions 0..51,
        # M at 64..115, idx-0 dummies elsewhere)] -> 384 rows, all valid.
        pofb = {int(bands[gi]): gi for gi in (0, 2, 4, 6)}
        mofb = {int(bands[gi]): gi for gi in (1, 3, 5, 7)}
        gf = np.zeros((NU_PER_CORE, 4, 384), np.int64)
        for j in range(NU_PER_CORE):
            for band in range(4):
                giP, giM = pofb[band], mofb[band]
                gf[j, band, 0:128] = gidx[j, giP, 0:128]
                gf[j, band, 128:256] = gidx[j, giM, 0:128]
                blk = np.zeros(128, np.int64)
                blk[0:52] = gidx[j, giP, 128:180]
                blk[64:116] = gidx[j, giM, 128:180]
                gf[j, band, 256:384] = blk
        # wrap idx: r -> [r%16, r//16], replicate to 128 partitions
        gw = np.zeros((NU_PER_CORE, 4, 16, 24), np.int16)
        for j in range(NU_PER_CORE):
            for band in range(4):
                for r in range(384):
                    gw[j, band, r % 16, r // 16] = gf[j, band, r]
        GIs.append(np.tile(gw, (1, 1, 8, 1)).astype(np.int16))
        merges.append(mspec)

    # irfft matrix [257, 512] fp64->fp32: h[t] = sum_f IRm[f, t] * k[f]
    tt = np.arange(T)
    ff = np.arange(T // 2 + 1)
    IRm = 2.0 * np.cos(2 * np.pi * np.outer(ff, tt) / T) / T
    IRm[0] *= 0.5
    IRm[T // 2] *= 0.5
    IRm = IRm.astype(np.float32)

    # circulant build offsets: C[s, t] = h2[512 - s + t]; per s-chunk [128,1]
    coffs = np.zeros((4, 128, 1), np.int32)
    for ch in range(4):
        for p in range(128):
            coffs[ch, p, 0] = 512 - (128 * ch + p)

    _plan_cache = dict(
        inv_a=inv_a, inv_s=inv_s, signs=signs, bands=bands,
        gT=gT, gP=gP, core_nus=core_nus,
        WAs=WAs, WBs=WBs, GIs=GIs, merges=merges,
        IRm=IRm, coffs=coffs,
    )
    return _plan_cache


def host_reference_from_plan(sinos, kern):
    """Numpy simulation of the EXACT device pipeline (bf16 quantization
    included) for validating the plan tables. Returns [B,256,256] fp32."""
    plan = build_plan()
    bf = ml_dtypes.bfloat16
    h = (plan["IRm"].astype(np.float64).T @ kern.astype(np.float64))  # [512]
    h2 = np.concatenate([h, h])
    Cm = np.zeros((T, T))
    for s in range(T):
        Cm[s] = h2[512 - s:1024 - s]
    Cm16 = Cm.astype(bf).astype(np.float64)
    sin16 = sinos.astype(bf).astype(np.float64)
    filt = np.einsum('bps,st->bpt', sin16, Cm16)
    filt16 = filt.astype(bf)  # DRAM filt [phi, t, b] bf16
    filtf = filt16.astype(np.float64)

    out = np.zeros((sinos.shape[0], H, W))
    for c in range(NCORE):
        WA = plan["WAs"][c].astype(np.float64)
        WB = plan["WBs"][c].astype(np.float64)
        for j, tau in enumerate(plan["core_nus"][c]):
            for gi in range(NG):
                s = plan["signs"][gi]
                aa = plan["inv_a"][gi, :NCLS]
                m = plan["gT"][gi][tau]
                pm = plan["gP"][gi]
                acc = np.zeros((sinos.shape[0], TS * TS))
                ti, tj = divmod(int(tau), NT)
                iiv = np.arange(TS, dtype=np.float64)
                b4 = np.zeros(NCLS, np.int64)
                for cc in range(NCLS):
                    Ac, Bcc, Ccc = _uABC(cc)
                    u = Ac * (ti * TS + iiv[:, None]) + Bcc * (tj * TS + iiv[None, :]) + Ccc
                    bb = (int(np.floor(u.min())) // 4) * 4
                    b4[cc] = np.clip(bb, 0, 476)
                mb = b4 if s == 1 else 476 - b4
                for cc in range(NCLS):
                    g = filtf[:, aa[cc], mb[cc]:mb[cc] + KWIN]     # [B, 36]
                    if s == -1:
                        g = g[:, ::-1]   # member row k reads tap mb + (35-k)
                    Wk = (WA[j, cc] if cc < 128 else WB[j, cc - 128])  # [36, 256]
                    acc += g @ Wk
                accp = np.zeros_like(acc)
                accp[:, pm] = acc
                mi, mj = divmod(int(m), NT)
                out[:, mi*TS:(mi+1)*TS, mj*TS:(mj+1)*TS] += \
                    accp.reshape(-1, TS, TS)
    return out.astype(np.float32)


# ======================================================================
# Device program
# ======================================================================
NROWS_GATHER = 23032          # max gather row index + 1 (per 180-angle band)
BAND_ELEMS = NCLS * T * B     # 2,949,120 elements per angle band
FILT_ELEMS = PHI * T * B      # 11,796,480
GROW = GWIN * B               # 1152 elements per gathered row
GI_P = (0, 2, 4, 6)           # sign +1 members (lhsT tap = k)
GI_M = (1, 3, 5, 7)           # sign -1 members (lhsT tap = KWIN-1-k)
GI_ORDER = GI_P + GI_M        # oslots slot -> gi

_nc_cache = None

def _build_nc():
    global _nc_cache
    if _nc_cache is not None:
        return _nc_cache
    import concourse.bass as bass
    import concourse.bacc as bacc
    import concourse.mybir as mybir
    import concourse.tile as tile

    plan = build_plan()
    signs = plan["signs"]; bands = plan["bands"]
    assert all(signs[gi] == 1 for gi in GI_P)
    assert all(signs[gi] == -1 for gi in GI_M)
    bf = mybir.dt.bfloat16
    f32 = mybir.dt.float32

    nc = bacc.Bacc(None, target_bir_lowering=False)
    sinoT = nc.dram_tensor("sinoT", [128, 4, PHI * B], bf, kind="ExternalInput")
    kern = nc.dram_tensor("kern", [384, 1], f32, kind="ExternalInput")
    irm = nc.dram_tensor("irm", [384, 512], f32, kind="ExternalInput")
    coffs = nc.dram_tensor("coffs", [4, 128, 1], mybir.dt.int32, kind="ExternalInput")
    wa = nc.dram_tensor("wa", [NU_PER_CORE, 128, KWIN, 256], bf, kind="ExternalInput")
    wb = nc.dram_tensor("wb", [NU_PER_CORE, 52, KWIN, 256], bf, kind="ExternalInput")
    gidx = nc.dram_tensor("gidx", [NU_PER_CORE, 4, 128, 24], mybir.dt.int16,
                          kind="ExternalInput")
    oslots = nc.dram_tensor("oslots", [NU_PER_CORE, NG, B, 256], bf,
                            kind="ExternalOutput")
    h2d = nc.dram_tensor("h2d", [1024, 1], f32)
    filt = nc.dram_tensor("filt", [FILT_ELEMS + 256], bf)
    ftens = filt.tensor if hasattr(filt, "tensor") else filt

    with tile.TileContext(nc) as tc:
        # ---------------- filter phase (replicated, all 720 angles) -----
        with (tc.tile_pool(name="fsb", bufs=1) as fsb,
              tc.tile_pool(name="fst", bufs=4) as fst,
              tc.tile_pool(name="fwork", bufs=4) as fwork,
              tc.tile_pool(name="fps", bufs=2, space="PSUM") as fps):
            # F0: h = IRm.T @ kern
            hps = fps.tile([1, 512], f32, tag="hps")
            for ch, (k0, ksz) in enumerate(((0, 128), (128, 128), (256, 128))):
                kt = fsb.tile([ksz, 1], f32, tag=f"kt{ch}")
                nc.sync.dma_start(kt[:], kern[k0:k0 + ksz, :])
                irt = fsb.tile([ksz, 512], f32, tag=f"irt{ch}")
                nc.sync.dma_start(irt[:], irm[k0:k0 + ksz, :])
                nc.tensor.matmul(hps[:], lhsT=kt[:], rhs=irt[:],
                                 start=(ch == 0), stop=(ch == 2))
            h2sb = fsb.tile([1, 1024], f32, tag="h2sb")
            nc.vector.tensor_copy(h2sb[:, 0:512], hps[:])
            nc.vector.tensor_copy(h2sb[:, 512:1024], hps[:])
            nc.sync.dma_start(h2d[:].rearrange("a b -> b a"), h2sb[:])

            # F1: circulant chunks C[ch] = h2[512 - s + t], bf16
            csb = []
            for ch in range(4):
                co = fsb.tile([128, 1], mybir.dt.int32, tag=f"co{ch}")
                nc.sync.dma_start(co[:], coffs[ch])
                cf = fsb.tile([128, 512], f32, tag=f"cf{ch}")
                nc.gpsimd.indirect_dma_start(
                    out=cf[:], out_offset=None, in_=h2d[:],
                    in_offset=bass.IndirectOffsetOnAxis(ap=co[:], axis=0))
                cb = fsb.tile([128, 512], bf, tag=f"cb{ch}")
                nc.vector.tensor_copy(cb[:], cf[:])
                csb.append(cb)

            # F2: filtered (transposed): psum[t, (phi,b)] = C[s,t].T @ sinoT
            # 1024-wide column blocks; one fused load and one fused store
            # per block to keep the DMA instruction count low.
            CSZ = 1024
            for cs in range(0, PHI * B, CSZ):
                csz = min(CSZ, PHI * B - cs)
                nphi = csz // B
                st = fst.tile([128, 4, CSZ], bf, tag="st")
                nc.sync.dma_start(st[:, :, :csz], sinoT[:, :, cs:cs + csz])
                fb = fwork.tile([128, 4, CSZ], bf, tag="fb")
                for tch in range(4):
                    fp = fps.tile([128, CSZ], f32, tag="fp")
                    for h0 in range(0, csz, 512):
                        hsz = min(512, csz - h0)
                        for ch in range(4):
                            for soff in (0, 64):
                                nc.tensor.matmul(
                                    fp[soff:soff + 64, h0:h0 + hsz],
                                    lhsT=csb[ch][:, tch * 128 + soff:
                                                 tch * 128 + soff + 64],
                                    rhs=st[:, ch, h0:h0 + hsz],
                                    start=(ch == 0), stop=(ch == 3),
                                    tile_position=(0, soff))
                    nc.vector.tensor_copy(fb[:, tch, :csz], fp[:, :csz])
                # store: partition p = t (tch*128+p); free = (phi_rel, b)
                phi0 = cs // B
                for tch in range(4):
                    out_ap = bass.AP(ftens, phi0 * (T * B) + tch * 128 * B,
                                     [[B, 128], [T * B, nphi], [1, B]])
                    nc.sync.dma_start(
                        out_ap,
                        fb[:, tch, :csz].rearrange("p (f b) -> p f b", b=B))

        # ---------------- backprojection ----------------
        with (tc.tile_pool(name="bsb", bufs=1) as bsb,
              tc.tile_pool(name="bg", bufs=2) as bg,
              tc.tile_pool(name="bg2", bufs=2) as bg2,
              tc.tile_pool(name="bw", bufs=2) as bw,
              tc.tile_pool(name="bst", bufs=2) as bst,
              tc.tile_pool(name="bps", bufs=2, space="PSUM") as bps):
            idxsb = bsb.tile([128, NU_PER_CORE * 4 * 24], mybir.dt.int16, tag="idx")
            nc.sync.dma_start(
                idxsb[:].rearrange("p (n g w) -> p n g w", n=NU_PER_CORE, g=4),
                gidx[:].transpose([2, 0, 1, 3]))
            otens = oslots.tensor if hasattr(oslots, "tensor") else oslots
            for nu in range(NU_PER_CORE):
                # one fused gather per (nu, band): free-slot 0 = P-member
                # chunk0, 1 = M-member chunk0, 2 = chunk1 pair (P classes on
                # partitions 0..51, M on 64..115, dummy idx-0 rows elsewhere)
                gts = []
                for band in range(4):
                    gt = bg.tile([128, 3, GWIN, B], bf, tag=f"gt{band}")
                    in_ap = bass.AP(ftens, band * BAND_ELEMS,
                                    [[128, NROWS_GATHER], [1, GROW]])
                    nc.gpsimd.dma_gather(
                        out_ap=gt[:].rearrange("p c k b -> p c (k b)"),
                        in_ap=in_ap,
                        idxs_ap=idxsb[:, (nu * 4 + band) * 24:
                                      (nu * 4 + band + 1) * 24],
                        num_idxs=384, num_idxs_reg=384,
                        elem_size=GROW, elem_step=128,
                        single_packet=False)
                    gts.append(gt)
                # rearrange band tiles into pair-contiguous layout so
                # same-half slot pairs form 64-wide lhsT slices (64-col
                # LDWEIGHTS measured at +3ns/MM vs 32-col; halves MM count)
                g2P = bg2.tile([128, 2, GWIN, 4, B], bf, tag="g2P")
                g2M = bg2.tile([128, 2, GWIN, 4, B], bf, tag="g2M")
                for g2, gis, fslot in ((g2P, GI_P, 0), (g2M, GI_M, 1)):
                    for s in range(4):
                        gt = gts[int(bands[gis[s]])]
                        nc.vector.tensor_copy(g2[:, 0, :, s, :], gt[:, fslot])
                        if fslot == 0:
                            nc.vector.tensor_copy(g2[0:52, 1, :, s, :],
                                                  gt[0:52, 2])
                        else:
                            nc.vector.tensor_copy(g2[64:116, 1, :, s, :],
                                                  gt[64:116, 2])
                wat = bw.tile([128, KWIN, 256], bf, tag="wa")
                nc.sync.dma_start(wat[:], wa[nu])
                # chunk-B weights at both partition bases used by the pair;
                # second copy moves SBUF->SBUF to spare contended HBM
                wbt = bw.tile([128, KWIN, 256], bf, tag="wb")
                nc.sync.dma_start(wbt[0:52], wb[nu])
                nc.sync.dma_start(wbt[64:116], wbt[0:52])
                psP = bps.tile([128, 256], f32, tag="psP")
                psM = bps.tile([128, 256], f32, tag="psM")
                for cchunk in range(2):
                    for k in range(KWIN):
                        km = KWIN - 1 - k
                        for ps, g2, half, ks in ((psP, g2P, 0, k),
                                                 (psM, g2M, 1, km)):
                            for p in range(2):
                                if cchunk == 0:
                                    lhs = g2[:, 0, ks, 2 * p:2 * p + 2, :]
                                    rhs = wat[:, k, :]
                                    tp = (0, p * 64)
                                elif half == 0:
                                    lhs = g2[0:52, 1, ks, 2 * p:2 * p + 2, :]
                                    rhs = wbt[0:52, k, :]
                                    tp = (0, p * 64)
                                else:
                                    lhs = g2[64:116, 1, ks, 2 * p:2 * p + 2, :]
                                    rhs = wbt[64:116, k, :]
                                    tp = (64, p * 64)
                                nc.tensor.matmul(
                                    ps[p * 64:(p + 1) * 64, :],
                                    lhsT=lhs, rhs=rhs,
                                    start=(cchunk == 0 and k == 0),
                                    stop=(cchunk == 1 and k == KWIN - 1),
                                    tile_position=tp)
                for half, ps in ((0, psP), (1, psM)):
                    stg = bst.tile([128, 256], bf, tag="stg")
                    nc.vector.tensor_copy(stg[:], ps[:])
                    out_ap = bass.AP(otens,
                                     nu * NG * B * 256 + half * 4 * B * 256,
                                     [[256, 128], [1, 256]])
                    nc.sync.dma_start(out_ap, stg[:])
    nc.compile()
    _nc_cache = nc
    return nc


def _prep_inputs(sinos, kern_in):
    plan = build_plan()
    bf = ml_dtypes.bfloat16
    kern_t = np.zeros((384, 1), np.float32)
    kern_t[:257, 0] = np.asarray(kern_in, np.float32)
    irm_pad = np.zeros((384, 512), np.float32)
    irm_pad[:257] = plan["IRm"]
    # full sinogram, transposed to [t, (phi, b)], replicated on every core;
    # layout [t%128, t//128, (phi, b)] so each filter block loads in one DMA
    st = np.asarray(sinos, np.float64).transpose(2, 1, 0).reshape(T, PHI * B)
    st = st.reshape(4, 128, PHI * B).transpose(1, 0, 2).astype(bf)
    in_maps = []
    for c in range(NCORE):
        in_maps.append({
            "sinoT": st,
            "kern": kern_t,
            "irm": irm_pad,
            "coffs": plan["coffs"],
            "wa": plan["WAs"][c],
            "wb": plan["WBs"][c],
            "gidx": plan["GIs"][c].reshape(NU_PER_CORE, 4, 128, 24),
        })
    return in_maps


def _merge_outputs(results):
    plan = build_plan()
    out = np.zeros((B, H, W), np.float64)
    for c in range(NCORE):
        slots = results[c]["oslots"].astype(np.float64)   # [16, 8, 32, 256]
        for j in range(NU_PER_CORE):
            for idx, gi in enumerate(GI_ORDER):
                m = int(plan["merges"][c][j, gi, 0])
                pm = plan["gP"][gi]
                accp = np.zeros((B, TS * TS))
                accp[:, pm] = slots[j, idx]
                mi, mj = divmod(m, NT)
                out[:, mi * TS:(mi + 1) * TS, mj * TS:(mj + 1) * TS] += \
                    accp.reshape(B, TS, TS)
    return out.astype(np.float32)


def kernel(sinos, kernel):
    from concourse.bass_utils import run_bass_kernel_spmd
    sinos = np.asarray(sinos)
    kern_in = np.asarray(kernel)
    nc = _build_nc()
    in_maps = _prep_inputs(sinos, kern_in)
    res = run_bass_kernel_spmd(nc, in_maps, list(range(NCORE)))
    return _merge_outputs(res.results)



# revision 15
# speedup vs baseline: 1.0390x; 1.0390x over previous
"""Trainium2 Bass kernel for filtered backprojection (FBP).

reference semantics:
    filtered = irfft(rfft(sinos, axis=-1) * kernel, n=512, axis=-1)
    out[b,i,j] = sum_phi lerp(filtered[b,phi,:], u(phi,i,j)) * DPHI
with u affine in (i,j) per angle.

Device pipeline (8 NeuronCores, SPMD, no collectives):
  F0  h = irfft(kernel) via small matmuls against a host irfft matrix
  F1  circulant C[s,t] = h[(t-s)%512] built via per-partition indirect DMA
  F2  filter (replicated on every core): filtered rows = sinoT.T @ C
      (bf16 matmuls) over all 720 angles, chunked 512 cols at a time,
      written to local DRAM as filt[phi, t, b] bf16.
  B   backprojection: image in 16x16 tiles; per (angle,tile) only a 36-wide
      detector window contributes. D4 symmetry (8 exact pixel-grid
      symmetries) dedups weight blocks 8x. Per canonical tile: the 8
      member tiles' windows are gathered into two sign-grouped tiles
      (4 slots each); matmuls use 32-wide lhsT strips via tile_position
      (32-col LDWEIGHTS pipelines for free on this toolchain; 128-col
      does not). Final 4-way sigma-permuted merge happens on host.

Weights/idx tables are pure geometry -> precomputed on host in fp64.
"""
import numpy as np
import ml_dtypes

# ---------------- geometry constants ----------------
PHI, T, H, W = 720, 512, 256, 256
RHO = float(np.sqrt(2.0))
DPHI = float(np.pi) / PHI
DT = 2.0 * RHO / T
T0 = -RHO + 0.5 * DT
DX = 2.0 / H
TS, NT = 16, 16            # tile size / tiles per side
KWIN = 36                  # weight k-window (taps per (class,tile))
GWIN = 36                  # gathered k-window (36*32 bf16 = 2304B, 256B-aligned)
NCLS = 180                 # D4 angle classes
NG = 8                     # group size
NCORE = 8
B = 32
NU_PER_CORE = 16           # canonical-tile units per core

# ---------------- D4 group tables ----------------
def _mats():
    out = []
    for swap in (False, True):
        for sx in (1, -1):
            for sy in (1, -1):
                if not swap:
                    out.append(np.array([[sx, 0], [0, sy]]))
                else:
                    out.append(np.array([[0, sx], [sy, 0]]))
    return out

MATS = _mats()

def _angle_dirs():
    th = (np.arange(PHI) + 0.5) * DPHI
    return np.stack([np.cos(th), np.sin(th)], axis=1)

def _angle_maps():
    dirs = _angle_dirs()
    amap = np.zeros((NG, PHI), np.int64)
    smap = np.zeros((NG, PHI), np.int64)
    for gi, M in enumerate(MATS):
        v = dirs @ M          # row a: M^T d(a)
        dots = v @ dirs.T     # [a, a']
        ip = np.argmax(dots, axis=1)
        im = np.argmin(dots, axis=1)
        for a in range(PHI):
            if dots[a, ip[a]] > 1 - 1e-9:
                amap[gi, a], smap[gi, a] = ip[a], 1
            elif dots[a, im[a]] < -1 + 1e-9:
                amap[gi, a], smap[gi, a] = im[a], -1
            else:
                raise AssertionError((gi, a))
    inv_a = np.zeros_like(amap); inv_s = np.zeros_like(smap)
    for gi in range(NG):
        inv_a[gi, amap[gi]] = np.arange(PHI)
        inv_s[gi, amap[gi]] = smap[gi]
    return amap, smap, inv_a, inv_s

def _pixel_map(M):
    def f(i, j):
        def comp(row, i, j):
            if row[0] == 1:  return i
            if row[0] == -1: return 255 - i
            if row[1] == 1:  return j
            return 255 - j
        return comp(M[0], i, j), comp(M[1], i, j)
    return f

def _tile_pixel_maps():
    gT, gP = [], []
    for M in MATS:
        f = _pixel_map(M)
        tm = np.zeros(NT * NT, np.int64)
        pm = np.zeros(TS * TS, np.int64)
        for ti in range(NT):
            for tj in range(NT):
                a1 = f(ti * TS, tj * TS)
                a2 = f(ti * TS + TS - 1, tj * TS + TS - 1)
                tm[ti * NT + tj] = (min(a1[0], a2[0]) // TS) * NT + min(a1[1], a2[1]) // TS
        for pi in range(TS):
            for pj in range(TS):
                i_, j_ = f(pi, pj)
                pm[pi * TS + pj] = (i_ % TS) * TS + (j_ % TS)
        gT.append(tm); gP.append(pm)
    return gT, gP

def _uABC(a):
    th = (np.asarray(a, np.float64) + 0.5) * DPHI
    A = DX * np.cos(th) / DT
    Bc = DX * np.sin(th) / DT
    C = ((-1 + 0.5 * DX) * (np.cos(th) + np.sin(th)) - T0) / DT
    return A, Bc, C

_plan_cache = None

def build_plan():
    global _plan_cache
    if _plan_cache is not None:
        return _plan_cache
    amap, smap, inv_a, inv_s = _angle_maps()
    gT, gP = _tile_pixel_maps()
    rho_i = next(gi for gi, M in enumerate(MATS)
                 if M[0][0] == -1 and M[0][1] == 0 and M[1][1] == -1)
    assert all(amap[rho_i, a] == a and smap[rho_i, a] == -1 for a in range(PHI))

    # per-g uniform sign + band of the inverse angle map over classes 0..179
    bands = np.zeros(NG, np.int64)
    signs = np.zeros(NG, np.int64)
    for gi in range(NG):
        aa = inv_a[gi, :NCLS]
        ss = inv_s[gi, :NCLS]
        assert (ss == ss[0]).all(), gi
        bd = aa // NCLS
        assert (bd == bd[0]).all(), (gi, np.unique(bd))
        bands[gi] = bd[0]
        signs[gi] = ss[0]

    # tile orbits under D4; canonical taus: tau < rho180(tau)
    rho_t = gT[rho_i]
    tseen = np.zeros(NT * NT, bool)
    orbits = []
    for t in range(NT * NT):
        if tseen[t]: continue
        tiles = []
        for gi in range(NG):
            tt = gT[gi][t]
            if not tseen[tt]:
                tseen[tt] = True
                tiles.append(tt)
        orbits.append(tiles)
    canon = [[t for t in tiles if t < rho_t[t]] for tiles in orbits]
    big = [i for i, o in enumerate(orbits) if len(o) == 8]
    small = [i for i, o in enumerate(orbits) if len(o) == 4]
    assert len(big) == 28 and len(small) == 8, (len(big), len(small))
    core_nus = []   # per core: list of 16 canonical tile ids
    for c in range(7):
        nus = []
        for oi in big[c * 4:(c + 1) * 4]:
            nus.extend(canon[oi])
        assert len(nus) == 16
        core_nus.append(nus)
    nus7 = []
    for oi in small:
        nus7.extend(canon[oi])
    assert len(nus7) == 16
    core_nus.append(nus7)

    # per (class, tile): fp64 window base (4-aligned), weights
    ii = np.arange(TS, dtype=np.float64)
    def u_tile(a, tile):
        ti, tj = divmod(int(tile), NT)
        A, Bc, C = _uABC(a)
        return (A * (ti * TS + ii[:, None]) + Bc * (tj * TS + ii[None, :]) + C)

    def base4_of(a, tile):
        u = u_tile(a, tile)
        b = int(np.floor(u.min()))
        b4 = (b // 4) * 4
        return int(np.clip(b4, 0, 476))

    # W tables per core: WA [16,128,KWIN,256], WB [16,52,KWIN,256] bf16
    # gather idx per core: [16, 8, 128, 12] int16 (wrapped, replicated)
    # merge spec per core: [16, 8] -> (tile m, sigma gi)  (m from gT)
    kk = np.arange(KWIN, dtype=np.float64)
    WAs, WBs, GIs, merges = [], [], [], []
    for c in range(NCORE):
        Wfull = np.zeros((NU_PER_CORE, NCLS, KWIN, TS * TS), np.float64)
        gidx = np.full((NU_PER_CORE, NG, 192), -1, np.int64)
        mspec = np.zeros((NU_PER_CORE, NG, 2), np.int64)
        for j, tau in enumerate(core_nus[c]):
            U = np.stack([u_tile(cc, tau).reshape(-1) for cc in range(NCLS)])  # [180,256]
            b4 = np.array([base4_of(cc, tau) for cc in range(NCLS)])           # [180]
            rel = U[:, None, :] - b4[:, None, None] - kk[None, :, None]
            Wk = np.maximum(0.0, 1.0 - np.abs(rel))
            tap = b4[:, None] + kk[None, :]
            Wk[(tap < 0) | (tap >= T), :] = 0.0
            Wfull[j] = Wk * DPHI
            for gi in range(NG):
                s = signs[gi]
                aa = inv_a[gi, :NCLS]          # member angles [180]
                arel = aa % NCLS
                mb = b4 if s == 1 else 476 - b4
                idxv = (arel * T + mb) // 4
                assert (idxv * 4 == arel * T + mb).all()
                assert idxv.max() < 32768
                gidx[j, gi, :NCLS] = idxv
                mspec[j, gi] = (gT[gi][tau], gi)
        WAs.append(Wfull[:, :128].astype(ml_dtypes.bfloat16))
        WBs.append(Wfull[:, 128:].astype(ml_dtypes.bfloat16))
        # fused per-(nu, band) gather rows: [P-member chunk0 (128),
        # M-member chunk0 (128), chunk1 pair (P at partitions 0..51,
        # M at 64..115, idx-0 dummies elsewhere)] -> 384 rows, all valid.
        pofb = {int(bands[gi]): gi for gi in (0, 2, 4, 6)}
        mofb = {int(bands[gi]): gi for gi in (1, 3, 5, 7)}
        gf = np.zeros((NU_PER_CORE, 4, 384), np.int64)
        for j in range(NU_PER_CORE):
            for band in range(4):
                giP, giM = pofb[band], mofb[band]
                gf[j, band, 0:128] = gidx[j, giP, 0:128]
                gf[j, band, 128:256] = gidx[j, giM, 0:128]
                blk = np.zeros(128, np.int64)
                blk[0:52] = gidx[j, giP, 128:180]
                blk[64:116] = gidx[j, giM, 128:180]
                gf[j, band, 256:384] = blk
        # wrap idx: r -> [r%16, r//16], replicate to 128 partitions
        gw = np.zeros((NU_PER_CORE, 4, 16, 24), np.int16)
        for j in range(NU_PER_CORE):
            for band in range(4):
                for r in range(384):
                    gw[j, band, r % 16, r // 16] = gf[j, band, r]
        GIs.append(np.tile(gw, (1, 1, 8, 1)).astype(np.int16))
        merges.append(mspec)

    # irfft matrix [257, 512] fp64->fp32: h[t] = sum_f IRm[f, t] * k[f]
    tt = np.arange(T)
    ff = np.arange(T // 2 + 1)
    IRm = 2.0 * np.cos(2 * np.pi * np.outer(ff, tt) / T) / T
    IRm[0] *= 0.5
    IRm[T // 2] *= 0.5
    IRm = IRm.astype(np.float32)

    # circulant build offsets: C[s, t] = h2[512 - s + t]; per s-chunk [128,1]
    coffs = np.zeros((4, 128, 1), np.int32)
    for ch in range(4):
        for p in range(128):
            coffs[ch, p, 0] = 512 - (128 * ch + p)

    pofb = {int(bands[gi]): gi for gi in (0, 2, 4, 6)}
    mofb = {int(bands[gi]): gi for gi in (1, 3, 5, 7)}
    slot_gis = [pofb[b] for b in range(4)] + [mofb[b] for b in range(4)]
    _plan_cache = dict(
        inv_a=inv_a, inv_s=inv_s, signs=signs, bands=bands,
        gT=gT, gP=gP, core_nus=core_nus,
        WAs=WAs, WBs=WBs, GIs=GIs, merges=merges,
        IRm=IRm, coffs=coffs, slot_gis=slot_gis,
    )
    return _plan_cache


def host_reference_from_plan(sinos, kern):
    """Numpy simulation of the EXACT device pipeline (bf16 quantization
    included) for validating the plan tables. Returns [B,256,256] fp32."""
    plan = build_plan()
    bf = ml_dtypes.bfloat16
    h = (plan["IRm"].astype(np.float64).T @ kern.astype(np.float64))  # [512]
    h2 = np.concatenate([h, h])
    Cm = np.zeros((T, T))
    for s in range(T):
        Cm[s] = h2[512 - s:1024 - s]
    Cm16 = Cm.astype(bf).astype(np.float64)
    sin16 = sinos.astype(bf).astype(np.float64)
    filt = np.einsum('bps,st->bpt', sin16, Cm16)
    filt16 = filt.astype(bf)  # DRAM filt [phi, t, b] bf16
    filtf = filt16.astype(np.float64)

    out = np.zeros((sinos.shape[0], H, W))
    for c in range(NCORE):
        WA = plan["WAs"][c].astype(np.float64)
        WB = plan["WBs"][c].astype(np.float64)
        for j, tau in enumerate(plan["core_nus"][c]):
            for gi in range(NG):
                s = plan["signs"][gi]
                aa = plan["inv_a"][gi, :NCLS]
                m = plan["gT"][gi][tau]
                pm = plan["gP"][gi]
                acc = np.zeros((sinos.shape[0], TS * TS))
                ti, tj = divmod(int(tau), NT)
                iiv = np.arange(TS, dtype=np.float64)
                b4 = np.zeros(NCLS, np.int64)
                for cc in range(NCLS):
                    Ac, Bcc, Ccc = _uABC(cc)
                    u = Ac * (ti * TS + iiv[:, None]) + Bcc * (tj * TS + iiv[None, :]) + Ccc
                    bb = (int(np.floor(u.min())) // 4) * 4
                    b4[cc] = np.clip(bb, 0, 476)
                mb = b4 if s == 1 else 476 - b4
                for cc in range(NCLS):
                    g = filtf[:, aa[cc], mb[cc]:mb[cc] + KWIN]     # [B, 36]
                    if s == -1:
                        g = g[:, ::-1]   # member row k reads tap mb + (35-k)
                    Wk = (WA[j, cc] if cc < 128 else WB[j, cc - 128])  # [36, 256]
                    acc += g @ Wk
                accp = np.zeros_like(acc)
                accp[:, pm] = acc
                mi, mj = divmod(int(m), NT)
                out[:, mi*TS:(mi+1)*TS, mj*TS:(mj+1)*TS] += \
                    accp.reshape(-1, TS, TS)
    return out.astype(np.float32)


# ======================================================================
# Device program
# ======================================================================
NROWS_GATHER = 23032          # max gather row index + 1 (per 180-angle band)
BAND_ELEMS = NCLS * T * B     # 2,949,120 elements per angle band
FILT_ELEMS = PHI * T * B      # 11,796,480
GROW = GWIN * B               # 1152 elements per gathered row
GI_P = (0, 2, 4, 6)           # sign +1 members (lhsT tap = k)
GI_M = (1, 3, 5, 7)           # sign -1 members (lhsT tap = KWIN-1-k)
GI_ORDER = GI_P + GI_M        # oslots slot -> gi

_nc_cache = {}

# device-program variants (A/B tunables)
CFG = dict(f2_m128=True, bp_m128=True, legacy_bp=True,
           no_f2_mm=False, no_f2_store=False, no_f2_ld=False,
           no_gather=False, no_g2copy=False, no_wdma=False,
           no_bp_mm=False, no_bp_store=False)

def _build_nc(phases=("filter", "backproj"), repeat=1, cfg=None):
    global _nc_cache
    cfg = dict(CFG if cfg is None else cfg)
    key = (tuple(phases), repeat, tuple(sorted(cfg.items())))
    if key in _nc_cache:
        return _nc_cache[key]
    phases = tuple(phases)
    import concourse.bass as bass
    import concourse.bacc as bacc
    import concourse.mybir as mybir
    import concourse.tile as tile

    plan = build_plan()
    signs = plan["signs"]; bands = plan["bands"]
    assert all(signs[gi] == 1 for gi in GI_P)
    assert all(signs[gi] == -1 for gi in GI_M)
    bf = mybir.dt.bfloat16
    f32 = mybir.dt.float32

    nc = bacc.Bacc(None, target_bir_lowering=False)
    sinoT = nc.dram_tensor("sinoT", [128, 4, PHI * B], bf, kind="ExternalInput")
    kern = nc.dram_tensor("kern", [384, 1], f32, kind="ExternalInput")
    irm = nc.dram_tensor("irm", [384, 512], f32, kind="ExternalInput")
    coffs = nc.dram_tensor("coffs", [4, 128, 1], mybir.dt.int32, kind="ExternalInput")
    wa = nc.dram_tensor("wa", [NU_PER_CORE, 128, KWIN, 256], bf, kind="ExternalInput")
    wb = nc.dram_tensor("wb", [NU_PER_CORE, 52, KWIN, 256], bf, kind="ExternalInput")
    gidx = nc.dram_tensor("gidx", [NU_PER_CORE, 4, 128, 24], mybir.dt.int16,
                          kind="ExternalInput")
    oslots = nc.dram_tensor("oslots", [NU_PER_CORE, NG, B, 256], bf,
                            kind="ExternalOutput")
    h2d = nc.dram_tensor("h2d", [1024, 1], f32)
    filt = nc.dram_tensor("filt", [FILT_ELEMS + 256], bf)
    ftens = filt.tensor if hasattr(filt, "tensor") else filt

    def _emit_filter(tc, sfx=""):
        with (tc.tile_pool(name="fsb" + sfx, bufs=1) as fsb,
              tc.tile_pool(name="fst" + sfx, bufs=4) as fst,
              tc.tile_pool(name="fwork" + sfx, bufs=4) as fwork,
              tc.tile_pool(name="fps" + sfx, bufs=2, space="PSUM") as fps):
            # F0: h = IRm.T @ kern
            hps = fps.tile([1, 512], f32, tag="hps")
            for ch, (k0, ksz) in enumerate(((0, 128), (128, 128), (256, 128))):
                kt = fsb.tile([ksz, 1], f32, tag=f"kt{ch}")
                nc.sync.dma_start(kt[:], kern[k0:k0 + ksz, :])
                irt = fsb.tile([ksz, 512], f32, tag=f"irt{ch}")
                nc.sync.dma_start(irt[:], irm[k0:k0 + ksz, :])
                nc.tensor.matmul(hps[:], lhsT=kt[:], rhs=irt[:],
                                 start=(ch == 0), stop=(ch == 2))
            h2sb = fsb.tile([1, 1024], f32, tag="h2sb")
            nc.vector.tensor_copy(h2sb[:, 0:512], hps[:])
            nc.vector.tensor_copy(h2sb[:, 512:1024], hps[:])
            nc.sync.dma_start(h2d[:].rearrange("a b -> b a"), h2sb[:])

            # F1: circulant chunks C[ch] = h2[512 - s + t], bf16
            csb = []
            for ch in range(4):
                co = fsb.tile([128, 1], mybir.dt.int32, tag=f"co{ch}")
                nc.sync.dma_start(co[:], coffs[ch])
                cf = fsb.tile([128, 512], f32, tag=f"cf{ch}")
                nc.gpsimd.indirect_dma_start(
                    out=cf[:], out_offset=None, in_=h2d[:],
                    in_offset=bass.IndirectOffsetOnAxis(ap=co[:], axis=0))
                cb = fsb.tile([128, 512], bf, tag=f"cb{ch}")
                nc.vector.tensor_copy(cb[:], cf[:])
                csb.append(cb)

            # F2: filtered (transposed): psum[t, (phi,b)] = C[s,t].T @ sinoT
            # 1024-wide column blocks; one fused load and one fused store
            # per block to keep the DMA instruction count low.
            CSZ = 1024
            for cs in range(0, PHI * B, CSZ):
                csz = min(CSZ, PHI * B - cs)
                nphi = csz // B
                st = fst.tile([128, 4, CSZ], bf, tag="st")
                if cfg["no_f2_ld"]:
                    nc.vector.memset(st[:].rearrange("p c x -> p (c x)"), 0.0)
                else:
                    nc.sync.dma_start(st[:, :, :csz], sinoT[:, :, cs:cs + csz])
                fb = fwork.tile([128, 4, CSZ], bf, tag="fb")
                for tch in range(4):
                    fp = fps.tile([128, CSZ], f32, tag="fp")
                    for h0 in range(0, csz, 512):
                        hsz = min(512, csz - h0)
                        if cfg["no_f2_mm"] and h0 == 0:
                            nc.vector.memset(fp[:, :csz], 0.0)
                        for ch in range(4):
                            if cfg["no_f2_mm"]:
                                continue
                            if cfg["f2_m128"]:
                                nc.tensor.matmul(
                                    fp[:, h0:h0 + hsz],
                                    lhsT=csb[ch][:, tch * 128:
                                                 tch * 128 + 128],
                                    rhs=st[:, ch, h0:h0 + hsz],
                                    start=(ch == 0), stop=(ch == 3),
                                    tile_position=(0, 0))
                                continue
                            for soff in (0, 64):
                                nc.tensor.matmul(
                                    fp[soff:soff + 64, h0:h0 + hsz],
                                    lhsT=csb[ch][:, tch * 128 + soff:
                                                 tch * 128 + soff + 64],
                                    rhs=st[:, ch, h0:h0 + hsz],
                                    start=(ch == 0), stop=(ch == 3),
                                    tile_position=(0, soff))
                    nc.vector.tensor_copy(fb[:, tch, :csz], fp[:, :csz])
                # store: partition p = t (tch*128+p); free = (phi_rel, b)
                phi0 = cs // B
                for tch in range(4):
                    if cfg["no_f2_store"]:
                        continue
                    out_ap = bass.AP(ftens, phi0 * (T * B) + tch * 128 * B,
                                     [[B, 128], [T * B, nphi], [1, B]])
                    nc.sync.dma_start(
                        out_ap,
                        fb[:, tch, :csz].rearrange("p (f b) -> p f b", b=B))

    # ---------------- backprojection ----------------
    # Direct-gather layout: per nu one slot-major tile
    # G[128, 4 slots(=bands), 3 chunks, GWIN, B]; each band-gather writes the
    # contiguous G[:, band] slice (chunk0 = P-member rows, chunk1 = M-member
    # rows, chunk2 = chunk1 P/M pair). lhsT APs stride across slots, so no
    # DVE repack copies are needed. PSUM slot order = band order (host merge
    # uses plan["slot_gis"]).
    def _emit_backproj_direct(tc, sfx=""):
        with (tc.tile_pool(name="bsb" + sfx, bufs=1) as bsb,
              tc.tile_pool(name="bg" + sfx, bufs=3) as bg,
              tc.tile_pool(name="bw" + sfx, bufs=2) as bw,
              tc.tile_pool(name="bst" + sfx, bufs=2) as bst,
              tc.tile_pool(name="bps" + sfx, bufs=2, space="PSUM") as bps):
            idxsb = bsb.tile([128, NU_PER_CORE * 4 * 24], mybir.dt.int16, tag="idx")
            nc.sync.dma_start(
                idxsb[:].rearrange("p (n g w) -> p n g w", n=NU_PER_CORE, g=4),
                gidx[:].transpose([2, 0, 1, 3]))
            otens = oslots.tensor if hasattr(oslots, "tensor") else oslots
            for nu in range(NU_PER_CORE):
                G = bg.tile([128, 4, 3, GWIN, B], bf, tag="G")
                for band in range(4):
                    in_ap = bass.AP(ftens, band * BAND_ELEMS,
                                    [[128, NROWS_GATHER], [1, GROW]])
                    if cfg["no_gather"]:
                        nc.vector.memset(
                            G[:, band].rearrange("p c k b -> p (c k b)"), 0.0)
                        continue
                    nc.gpsimd.dma_gather(
                        out_ap=G[:, band].rearrange("p c k b -> p c (k b)"),
                        in_ap=in_ap,
                        idxs_ap=idxsb[:, (nu * 4 + band) * 24:
                                      (nu * 4 + band + 1) * 24],
                        num_idxs=384, num_idxs_reg=384,
                        elem_size=GROW, elem_step=128,
                        single_packet=False)
                wat = bw.tile([128, KWIN, 256], bf, tag="wa")
                if cfg["no_wdma"]:
                    nc.vector.memset(wat[:].rearrange("p k x -> p (k x)"), 0.0)
                else:
                    nc.sync.dma_start(wat[:], wa[nu])
                wbt = bw.tile([128, KWIN, 256], bf, tag="wb")
                if cfg["no_wdma"]:
                    nc.vector.memset(wbt[:].rearrange("p k x -> p (k x)"), 0.0)
                else:
                    nc.sync.dma_start(wbt[0:52], wb[nu])
                    nc.sync.dma_start(wbt[64:116], wbt[0:52])
                psP = bps.tile([128, 256], f32, tag="psP")
                psM = bps.tile([128, 256], f32, tag="psM")
                if cfg["no_bp_mm"]:
                    nc.vector.memset(psP[:], 0.0)
                    nc.vector.memset(psM[:], 0.0)
                for cchunk in range(0 if cfg["no_bp_mm"] else 2):
                    for k in range(KWIN):
                        km = KWIN - 1 - k
                        for ps, creg, half, ks in ((psP, 0, 0, k),
                                                   (psM, 1, 1, km)):
                            if cfg["bp_m128"]:
                                if cchunk == 0:
                                    lhs = G[:, :, creg, ks, :]
                                    rhs = wat[:, k, :]
                                    tp = (0, 0)
                                elif half == 0:
                                    lhs = G[0:52, :, 2, k, :]
                                    rhs = wbt[0:52, k, :]
                                    tp = (0, 0)
                                else:
                                    lhs = G[64:116, :, 2, km, :]
                                    rhs = wbt[64:116, k, :]
                                    tp = (64, 0)
                                nc.tensor.matmul(
                                    ps[:, :],
                                    lhsT=lhs, rhs=rhs,
                                    start=(cchunk == 0 and k == 0),
                                    stop=(cchunk == 1 and k == KWIN - 1),
                                    tile_position=tp)
                                continue
                            for p in range(2):
                                if cchunk == 0:
                                    lhs = G[:, 2 * p:2 * p + 2, creg, ks, :]
                                    rhs = wat[:, k, :]
                                    tp = (0, p * 64)
                                elif half == 0:
                                    lhs = G[0:52, 2 * p:2 * p + 2, 2, k, :]
                                    rhs = wbt[0:52, k, :]
                                    tp = (0, p * 64)
                                else:
                                    lhs = G[64:116, 2 * p:2 * p + 2, 2, km, :]
                                    rhs = wbt[64:116, k, :]
                                    tp = (64, p * 64)
                                nc.tensor.matmul(
                                    ps[p * 64:(p + 1) * 64, :],
                                    lhsT=lhs, rhs=rhs,
                                    start=(cchunk == 0 and k == 0),
                                    stop=(cchunk == 1 and k == KWIN - 1),
                                    tile_position=tp)
                for half, ps in (() if cfg["no_bp_store"] else
                                 ((0, psP), (1, psM))):
                    stg = bst.tile([128, 256], bf, tag="stg")
                    nc.vector.tensor_copy(stg[:], ps[:])
                    out_ap = bass.AP(otens,
                                     nu * NG * B * 256 + half * 4 * B * 256,
                                     [[256, 128], [1, 256]])
                    nc.sync.dma_start(out_ap, stg[:])

    def _emit_backproj(tc, sfx=""):
        with (tc.tile_pool(name="bsb" + sfx, bufs=1) as bsb,
              tc.tile_pool(name="bg" + sfx, bufs=2) as bg,
              tc.tile_pool(name="bg2" + sfx, bufs=2) as bg2,
              tc.tile_pool(name="bw" + sfx, bufs=2) as bw,
              tc.tile_pool(name="bst" + sfx, bufs=2) as bst,
              tc.tile_pool(name="bps" + sfx, bufs=2, space="PSUM") as bps):
            idxsb = bsb.tile([128, NU_PER_CORE * 4 * 24], mybir.dt.int16, tag="idx")
            nc.sync.dma_start(
                idxsb[:].rearrange("p (n g w) -> p n g w", n=NU_PER_CORE, g=4),
                gidx[:].transpose([2, 0, 1, 3]))
            otens = oslots.tensor if hasattr(oslots, "tensor") else oslots
            for nu in range(NU_PER_CORE):
                # one fused gather per (nu, band): free-slot 0 = P-member
                # chunk0, 1 = M-member chunk0, 2 = chunk1 pair (P classes on
                # partitions 0..51, M on 64..115, dummy idx-0 rows elsewhere)
                gts = []
                for band in range(4):
                    gt = bg.tile([128, 3, GWIN, B], bf, tag=f"gt{band}")
                    in_ap = bass.AP(ftens, band * BAND_ELEMS,
                                    [[128, NROWS_GATHER], [1, GROW]])
                    if cfg["no_gather"]:
                        nc.vector.memset(
                            gt[:].rearrange("p c k b -> p (c k b)"), 0.0)
                        gts.append(gt)
                        continue
                    nc.gpsimd.dma_gather(
                        out_ap=gt[:].rearrange("p c k b -> p c (k b)"),
                        in_ap=in_ap,
                        idxs_ap=idxsb[:, (nu * 4 + band) * 24:
                                      (nu * 4 + band + 1) * 24],
                        num_idxs=384, num_idxs_reg=384,
                        elem_size=GROW, elem_step=128,
                        single_packet=False)
                    gts.append(gt)
                # rearrange band tiles into pair-contiguous layout so
                # same-half slot pairs form 64-wide lhsT slices (64-col
                # LDWEIGHTS measured at +3ns/MM vs 32-col; halves MM count)
                g2P = bg2.tile([128, 2, GWIN, 4, B], bf, tag="g2P")
                g2M = bg2.tile([128, 2, GWIN, 4, B], bf, tag="g2M")
                if cfg["no_g2copy"]:
                    for g2 in (g2P, g2M):
                        nc.vector.memset(
                            g2[:].rearrange("p c k s b -> p (c k s b)"), 0.0)
                for g2, gis, fslot in (() if cfg["no_g2copy"] else
                                       ((g2P, GI_P, 0), (g2M, GI_M, 1))):
                    for s in range(4):
                        gt = gts[int(bands[gis[s]])]
                        nc.vector.tensor_copy(g2[:, 0, :, s, :], gt[:, fslot])
                        if fslot == 0:
                            nc.vector.tensor_copy(g2[0:52, 1, :, s, :],
                                                  gt[0:52, 2])
                        else:
                            nc.vector.tensor_copy(g2[64:116, 1, :, s, :],
                                                  gt[64:116, 2])
                wat = bw.tile([128, KWIN, 256], bf, tag="wa")
                if cfg["no_wdma"]:
                    nc.vector.memset(wat[:].rearrange("p k x -> p (k x)"), 0.0)
                else:
                    nc.sync.dma_start(wat[:], wa[nu])
                # chunk-B weights at both partition bases used by the pair;
                # second copy moves SBUF->SBUF to spare contended HBM
                wbt = bw.tile([128, KWIN, 256], bf, tag="wb")
                if cfg["no_wdma"]:
                    nc.vector.memset(wbt[:].rearrange("p k x -> p (k x)"), 0.0)
                else:
                    nc.sync.dma_start(wbt[0:52], wb[nu])
                    nc.sync.dma_start(wbt[64:116], wbt[0:52])
                psP = bps.tile([128, 256], f32, tag="psP")
                psM = bps.tile([128, 256], f32, tag="psM")
                if cfg["no_bp_mm"]:
                    nc.vector.memset(psP[:], 0.0)
                    nc.vector.memset(psM[:], 0.0)
                for cchunk in range(0 if cfg["no_bp_mm"] else 2):
                    for k in range(KWIN):
                        km = KWIN - 1 - k
                        for ps, g2, half, ks in ((psP, g2P, 0, k),
                                                 (psM, g2M, 1, km)):
                            if cfg["bp_m128"]:
                                if cchunk == 0:
                                    lhs = g2[:, 0, ks, :, :]
                                    rhs = wat[:, k, :]
                                    tp = (0, 0)
                                elif half == 0:
                                    lhs = g2[0:52, 1, ks, :, :]
                                    rhs = wbt[0:52, k, :]
                                    tp = (0, 0)
                                else:
                                    lhs = g2[64:116, 1, ks, :, :]
                                    rhs = wbt[64:116, k, :]
                                    tp = (64, 0)
                                nc.tensor.matmul(
                                    ps[:, :],
                                    lhsT=lhs, rhs=rhs,
                                    start=(cchunk == 0 and k == 0),
                                    stop=(cchunk == 1 and k == KWIN - 1),
                                    tile_position=tp)
                                continue
                            for p in range(2):
                                if cchunk == 0:
                                    lhs = g2[:, 0, ks, 2 * p:2 * p + 2, :]
                                    rhs = wat[:, k, :]
                                    tp = (0, p * 64)
                                elif half == 0:
                                    lhs = g2[0:52, 1, ks, 2 * p:2 * p + 2, :]
                                    rhs = wbt[0:52, k, :]
                                    tp = (0, p * 64)
                                else:
                                    lhs = g2[64:116, 1, ks, 2 * p:2 * p + 2, :]
                                    rhs = wbt[64:116, k, :]
                                    tp = (64, p * 64)
                                nc.tensor.matmul(
                                    ps[p * 64:(p + 1) * 64, :],
                                    lhsT=lhs, rhs=rhs,
                                    start=(cchunk == 0 and k == 0),
                                    stop=(cchunk == 1 and k == KWIN - 1),
                                    tile_position=tp)
                for half, ps in (() if cfg["no_bp_store"] else
                                 ((0, psP), (1, psM))):
                    stg = bst.tile([128, 256], bf, tag="stg")
                    nc.vector.tensor_copy(stg[:], ps[:])
                    out_ap = bass.AP(otens,
                                     nu * NG * B * 256 + half * 4 * B * 256,
                                     [[256, 128], [1, 256]])
                    nc.sync.dma_start(out_ap, stg[:])

    with tile.TileContext(nc) as tc:
        for r in range(repeat):
            sfx = "" if repeat == 1 else f"r{r}"
            if "filter" in phases:
                _emit_filter(tc, sfx)
            if "backproj" in phases:
                if cfg["legacy_bp"]:
                    _emit_backproj(tc, sfx)
                else:
                    _emit_backproj_direct(tc, sfx)
    nc.compile()
    _nc_cache[key] = nc
    return nc


def _prep_inputs(sinos, kern_in):
    plan = build_plan()
    bf = ml_dtypes.bfloat16
    kern_t = np.zeros((384, 1), np.float32)
    kern_t[:257, 0] = np.asarray(kern_in, np.float32)
    irm_pad = np.zeros((384, 512), np.float32)
    irm_pad[:257] = plan["IRm"]
    # full sinogram, transposed to [t, (phi, b)], replicated on every core;
    # layout [t%128, t//128, (phi, b)] so each filter block loads in one DMA
    st = np.asarray(sinos, np.float64).transpose(2, 1, 0).reshape(T, PHI * B)
    st = st.reshape(4, 128, PHI * B).transpose(1, 0, 2).astype(bf)
    in_maps = []
    for c in range(NCORE):
        in_maps.append({
            "sinoT": st,
            "kern": kern_t,
            "irm": irm_pad,
            "coffs": plan["coffs"],
            "wa": plan["WAs"][c],
            "wb": plan["WBs"][c],
            "gidx": plan["GIs"][c].reshape(NU_PER_CORE, 4, 128, 24),
        })
    return in_maps


def _merge_outputs(results, slot_order=None):
    plan = build_plan()
    if slot_order is None:
        slot_order = GI_ORDER if CFG["legacy_bp"] else plan["slot_gis"]
    out = np.zeros((B, H, W), np.float64)
    for c in range(NCORE):
        slots = results[c]["oslots"].astype(np.float64)   # [16, 8, 32, 256]
        for j in range(NU_PER_CORE):
            for idx, gi in enumerate(slot_order):
                m = int(plan["merges"][c][j, gi, 0])
                pm = plan["gP"][gi]
                accp = np.zeros((B, TS * TS))
                accp[:, pm] = slots[j, idx]
                mi, mj = divmod(m, NT)
                out[:, mi * TS:(mi + 1) * TS, mj * TS:(mj + 1) * TS] += \
                    accp.reshape(B, TS, TS)
    return out.astype(np.float32)


def kernel(sinos, kernel):
    from concourse.bass_utils import run_bass_kernel_spmd
    sinos = np.asarray(sinos)
    kern_in = np.asarray(kernel)
    nc = _build_nc()
    in_maps = _prep_inputs(sinos, kern_in)
    res = run_bass_kernel_spmd(nc, in_maps, list(range(NCORE)))
    return _merge_outputs(res.results)



# revision 34
# speedup vs baseline: 1.3553x; 1.3045x over previous
"""Trainium2 Bass kernel for filtered backprojection (FBP).

reference semantics:
    filtered = irfft(rfft(sinos, axis=-1) * kernel, n=512, axis=-1)
    out[b,i,j] = sum_phi lerp(filtered[b,phi,:], u(phi,i,j)) * DPHI
with u affine in (i,j) per angle.

Device pipeline (8 NeuronCores, SPMD, no collectives):
  F0  h = irfft(kernel) via small matmuls against a host irfft matrix
  F1  circulant C[s,t] = h[(t-s)%512] built via per-partition indirect DMA
  F2  filter (replicated on every core): filtered rows = sinoT.T @ C
      (bf16 matmuls) over all 720 angles, chunked 512 cols at a time,
      written to local DRAM as filt[phi, t, b] bf16.
  B   backprojection: image in 16x16 tiles; per (angle,tile) only a 36-wide
      detector window contributes. D4 symmetry (8 exact pixel-grid
      symmetries) dedups weight blocks 8x. Per canonical tile: the 8
      member tiles' windows are gathered into two sign-grouped tiles
      (4 slots each); matmuls use full 128-wide lhsT (single LDWEIGHTS
      per MM -- halves PE instruction count vs the 64-col split and is
      bit-exact; measured neutral-to-slightly-better on HW).
      Final 4-way sigma-permuted merge happens on host.

CFG flags keep A/B variants selectable; defaults = shipping config.
wgen=True generates W on device from affine-u geometry (correct, kills
53MB/core of weight inputs) but measured 1.63x SLOWER on HW: the +2.3k
fine-grained cross-engine vector/act instructions cost ~0.7us each --
this machine is bound by per-instruction/sync overhead, not data path.

Weights/idx tables are pure geometry -> precomputed on host in fp64.
"""
import numpy as np
import ml_dtypes

# ---------------- geometry constants ----------------
PHI, T, H, W = 720, 512, 256, 256
RHO = float(np.sqrt(2.0))
DPHI = float(np.pi) / PHI
DT = 2.0 * RHO / T
T0 = -RHO + 0.5 * DT
DX = 2.0 / H
TS, NT = 16, 16            # tile size / tiles per side
KWIN = 36                  # weight k-window (taps per (class,tile))
GWIN = 36                  # gathered k-window (36*32 bf16 = 2304B, 256B-aligned)
NCLS = 180                 # D4 angle classes
NG = 8                     # group size
NCORE = 8
B = 32
NU_PER_CORE = 16           # canonical-tile units per core

# ---------------- D4 group tables ----------------
def _mats():
    out = []
    for swap in (False, True):
        for sx in (1, -1):
            for sy in (1, -1):
                if not swap:
                    out.append(np.array([[sx, 0], [0, sy]]))
                else:
                    out.append(np.array([[0, sx], [sy, 0]]))
    return out

MATS = _mats()

def _angle_dirs():
    th = (np.arange(PHI) + 0.5) * DPHI
    return np.stack([np.cos(th), np.sin(th)], axis=1)

def _angle_maps():
    dirs = _angle_dirs()
    amap = np.zeros((NG, PHI), np.int64)
    smap = np.zeros((NG, PHI), np.int64)
    for gi, M in enumerate(MATS):
        v = dirs @ M          # row a: M^T d(a)
        dots = v @ dirs.T     # [a, a']
        ip = np.argmax(dots, axis=1)
        im = np.argmin(dots, axis=1)
        for a in range(PHI):
            if dots[a, ip[a]] > 1 - 1e-9:
                amap[gi, a], smap[gi, a] = ip[a], 1
            elif dots[a, im[a]] < -1 + 1e-9:
                amap[gi, a], smap[gi, a] = im[a], -1
            else:
                raise AssertionError((gi, a))
    inv_a = np.zeros_like(amap); inv_s = np.zeros_like(smap)
    for gi in range(NG):
        inv_a[gi, amap[gi]] = np.arange(PHI)
        inv_s[gi, amap[gi]] = smap[gi]
    return amap, smap, inv_a, inv_s

def _pixel_map(M):
    def f(i, j):
        def comp(row, i, j):
            if row[0] == 1:  return i
            if row[0] == -1: return 255 - i
            if row[1] == 1:  return j
            return 255 - j
        return comp(M[0], i, j), comp(M[1], i, j)
    return f

def _tile_pixel_maps():
    gT, gP = [], []
    for M in MATS:
        f = _pixel_map(M)
        tm = np.zeros(NT * NT, np.int64)
        pm = np.zeros(TS * TS, np.int64)
        for ti in range(NT):
            for tj in range(NT):
                a1 = f(ti * TS, tj * TS)
                a2 = f(ti * TS + TS - 1, tj * TS + TS - 1)
                tm[ti * NT + tj] = (min(a1[0], a2[0]) // TS) * NT + min(a1[1], a2[1]) // TS
        for pi in range(TS):
            for pj in range(TS):
                i_, j_ = f(pi, pj)
                pm[pi * TS + pj] = (i_ % TS) * TS + (j_ % TS)
        gT.append(tm); gP.append(pm)
    return gT, gP

def _uABC(a):
    th = (np.asarray(a, np.float64) + 0.5) * DPHI
    A = DX * np.cos(th) / DT
    Bc = DX * np.sin(th) / DT
    C = ((-1 + 0.5 * DX) * (np.cos(th) + np.sin(th)) - T0) / DT
    return A, Bc, C

def _base4_of(a, tile):
    ti, tj = divmod(int(tile), NT)
    ii = np.arange(TS, dtype=np.float64)
    A, Bc, C = _uABC(a)
    u = A * (ti * TS + ii[:, None]) + Bc * (tj * TS + ii[None, :]) + C
    b = int(np.floor(u.min()))
    return int(np.clip((b // 4) * 4, 0, 476))


def _build_geo(core_nus_c):
    """Per-core W-generation geometry: [3, NU, 2, 128] fp32 rows (A, B, C2)
    with C2 = A*16ti + B*16tj + C - b4; chunk1 duplicates classes 128..179 at
    slots 0..51 and 64..115; invalid slots get C2=-5e4 (=> W=0)."""
    geo = np.zeros((3, NU_PER_CORE, 2, 128), np.float64)
    geo[2, :, :, :] = -5e4
    for j, tau in enumerate(core_nus_c):
        ti, tj = divmod(int(tau), NT)
        for c2 in range(2):
            for m in range(128):
                if c2 == 0:
                    cls = m
                else:
                    if m < 52:
                        cls = 128 + m
                    elif 64 <= m < 116:
                        cls = 128 + (m - 64)
                    else:
                        continue
                A, Bc, C = _uABC(cls)
                b4 = _base4_of(cls, tau)
                geo[0, j, c2, m] = A
                geo[1, j, c2, m] = Bc
                geo[2, j, c2, m] = A * 16 * ti + Bc * 16 * tj + C - b4
    return geo.astype(np.float32)


_plan_cache = None

def build_plan():
    global _plan_cache
    if _plan_cache is not None:
        return _plan_cache
    amap, smap, inv_a, inv_s = _angle_maps()
    gT, gP = _tile_pixel_maps()
    rho_i = next(gi for gi, M in enumerate(MATS)
                 if M[0][0] == -1 and M[0][1] == 0 and M[1][1] == -1)
    assert all(amap[rho_i, a] == a and smap[rho_i, a] == -1 for a in range(PHI))

    # per-g uniform sign + band of the inverse angle map over classes 0..179
    bands = np.zeros(NG, np.int64)
    signs = np.zeros(NG, np.int64)
    for gi in range(NG):
        aa = inv_a[gi, :NCLS]
        ss = inv_s[gi, :NCLS]
        assert (ss == ss[0]).all(), gi
        bd = aa // NCLS
        assert (bd == bd[0]).all(), (gi, np.unique(bd))
        bands[gi] = bd[0]
        signs[gi] = ss[0]

    # tile orbits under D4; canonical taus: tau < rho180(tau)
    rho_t = gT[rho_i]
    tseen = np.zeros(NT * NT, bool)
    orbits = []
    for t in range(NT * NT):
        if tseen[t]: continue
        tiles = []
        for gi in range(NG):
            tt = gT[gi][t]
            if not tseen[tt]:
                tseen[tt] = True
                tiles.append(tt)
        orbits.append(tiles)
    canon = [[t for t in tiles if t < rho_t[t]] for tiles in orbits]
    big = [i for i, o in enumerate(orbits) if len(o) == 8]
    small = [i for i, o in enumerate(orbits) if len(o) == 4]
    assert len(big) == 28 and len(small) == 8, (len(big), len(small))
    core_nus = []   # per core: list of 16 canonical tile ids
    for c in range(7):
        nus = []
        for oi in big[c * 4:(c + 1) * 4]:
            nus.extend(canon[oi])
        assert len(nus) == 16
        core_nus.append(nus)
    nus7 = []
    for oi in small:
        nus7.extend(canon[oi])
    assert len(nus7) == 16
    core_nus.append(nus7)

    # per (class, tile): fp64 window base (4-aligned), weights
    ii = np.arange(TS, dtype=np.float64)
    def u_tile(a, tile):
        ti, tj = divmod(int(tile), NT)
        A, Bc, C = _uABC(a)
        return (A * (ti * TS + ii[:, None]) + Bc * (tj * TS + ii[None, :]) + C)

    def base4_of(a, tile):
        u = u_tile(a, tile)
        b = int(np.floor(u.min()))
        b4 = (b // 4) * 4
        return int(np.clip(b4, 0, 476))

    # W tables per core: WA [16,128,KWIN,256], WB [16,52,KWIN,256] bf16
    # gather idx per core: [16, 8, 128, 12] int16 (wrapped, replicated)
    # merge spec per core: [16, 8] -> (tile m, sigma gi)  (m from gT)
    kk = np.arange(KWIN, dtype=np.float64)
    WAs, WBs, GIs, merges = [], [], [], []
    for c in range(NCORE):
        Wfull = np.zeros((NU_PER_CORE, NCLS, KWIN, TS * TS), np.float64)
        gidx = np.full((NU_PER_CORE, NG, 192), -1, np.int64)
        mspec = np.zeros((NU_PER_CORE, NG, 2), np.int64)
        for j, tau in enumerate(core_nus[c]):
            U = np.stack([u_tile(cc, tau).reshape(-1) for cc in range(NCLS)])  # [180,256]
            b4 = np.array([base4_of(cc, tau) for cc in range(NCLS)])           # [180]
            rel = U[:, None, :] - b4[:, None, None] - kk[None, :, None]
            Wk = np.maximum(0.0, 1.0 - np.abs(rel))
            tap = b4[:, None] + kk[None, :]
            Wk[(tap < 0) | (tap >= T), :] = 0.0
            Wfull[j] = Wk * DPHI
            for gi in range(NG):
                s = signs[gi]
                aa = inv_a[gi, :NCLS]          # member angles [180]
                arel = aa % NCLS
                mb = b4 if s == 1 else 476 - b4
                idxv = (arel * T + mb) // 4
                assert (idxv * 4 == arel * T + mb).all()
                assert idxv.max() < 32768
                gidx[j, gi, :NCLS] = idxv
                mspec[j, gi] = (gT[gi][tau], gi)
        WAs.append(Wfull[:, :128].astype(ml_dtypes.bfloat16))
        WBs.append(Wfull[:, 128:].astype(ml_dtypes.bfloat16))
        # fused per-(nu, band) gather rows: [P-member chunk0 (128),
        # M-member chunk0 (128), chunk1 pair (P at partitions 0..51,
        # M at 64..115, idx-0 dummies elsewhere)] -> 384 rows, all valid.
        pofb = {int(bands[gi]): gi for gi in (0, 2, 4, 6)}
        mofb = {int(bands[gi]): gi for gi in (1, 3, 5, 7)}
        gf = np.zeros((NU_PER_CORE, 4, 384), np.int64)
        for j in range(NU_PER_CORE):
            for band in range(4):
                giP, giM = pofb[band], mofb[band]
                gf[j, band, 0:128] = gidx[j, giP, 0:128]
                gf[j, band, 128:256] = gidx[j, giM, 0:128]
                blk = np.zeros(128, np.int64)
                blk[0:52] = gidx[j, giP, 128:180]
                blk[64:116] = gidx[j, giM, 128:180]
                gf[j, band, 256:384] = blk
        # wrap idx: r -> [r%16, r//16], replicate to 128 partitions
        gw = np.zeros((NU_PER_CORE, 4, 16, 24), np.int16)
        for j in range(NU_PER_CORE):
            for band in range(4):
                for r in range(384):
                    gw[j, band, r % 16, r // 16] = gf[j, band, r]
        GIs.append(np.tile(gw, (1, 1, 8, 1)).astype(np.int16))
        merges.append(mspec)

    # irfft matrix [257, 512] fp64->fp32: h[t] = sum_f IRm[f, t] * k[f]
    tt = np.arange(T)
    ff = np.arange(T // 2 + 1)
    IRm = 2.0 * np.cos(2 * np.pi * np.outer(ff, tt) / T) / T
    IRm[0] *= 0.5
    IRm[T // 2] *= 0.5
    IRm = IRm.astype(np.float32)

    # circulant build offsets: C[s, t] = h2[512 - s + t]; per s-chunk [128,1]
    coffs = np.zeros((4, 128, 1), np.int32)
    for ch in range(4):
        for p in range(128):
            coffs[ch, p, 0] = 512 - (128 * ch + p)

    pofb = {int(bands[gi]): gi for gi in (0, 2, 4, 6)}
    mofb = {int(bands[gi]): gi for gi in (1, 3, 5, 7)}
    slot_gis = [pofb[b] for b in range(4)] + [mofb[b] for b in range(4)]
    _plan_cache = dict(
        inv_a=inv_a, inv_s=inv_s, signs=signs, bands=bands,
        gT=gT, gP=gP, core_nus=core_nus,
        WAs=WAs, WBs=WBs, GIs=GIs, merges=merges,
        IRm=IRm, coffs=coffs, slot_gis=slot_gis,
    )
    return _plan_cache


def host_reference_from_plan(sinos, kern):
    """Numpy simulation of the EXACT device pipeline (bf16 quantization
    included) for validating the plan tables. Returns [B,256,256] fp32."""
    plan = build_plan()
    bf = ml_dtypes.bfloat16
    h = (plan["IRm"].astype(np.float64).T @ kern.astype(np.float64))  # [512]
    h2 = np.concatenate([h, h])
    Cm = np.zeros((T, T))
    for s in range(T):
        Cm[s] = h2[512 - s:1024 - s]
    Cm16 = Cm.astype(bf).astype(np.float64)
    sin16 = sinos.astype(bf).astype(np.float64)
    filt = np.einsum('bps,st->bpt', sin16, Cm16)
    filt16 = filt.astype(bf)  # DRAM filt [phi, t, b] bf16
    filtf = filt16.astype(np.float64)

    out = np.zeros((sinos.shape[0], H, W))
    for c in range(NCORE):
        WA = plan["WAs"][c].astype(np.float64)
        WB = plan["WBs"][c].astype(np.float64)
        for j, tau in enumerate(plan["core_nus"][c]):
            for gi in range(NG):
                s = plan["signs"][gi]
                aa = plan["inv_a"][gi, :NCLS]
                m = plan["gT"][gi][tau]
                pm = plan["gP"][gi]
                acc = np.zeros((sinos.shape[0], TS * TS))
                ti, tj = divmod(int(tau), NT)
                iiv = np.arange(TS, dtype=np.float64)
                b4 = np.zeros(NCLS, np.int64)
                for cc in range(NCLS):
                    Ac, Bcc, Ccc = _uABC(cc)
                    u = Ac * (ti * TS + iiv[:, None]) + Bcc * (tj * TS + iiv[None, :]) + Ccc
                    bb = (int(np.floor(u.min())) // 4) * 4
                    b4[cc] = np.clip(bb, 0, 476)
                mb = b4 if s == 1 else 476 - b4
                for cc in range(NCLS):
                    g = filtf[:, aa[cc], mb[cc]:mb[cc] + KWIN]     # [B, 36]
                    if s == -1:
                        g = g[:, ::-1]   # member row k reads tap mb + (35-k)
                    Wk = (WA[j, cc] if cc < 128 else WB[j, cc - 128])  # [36, 256]
                    acc += g @ Wk
                accp = np.zeros_like(acc)
                accp[:, pm] = acc
                mi, mj = divmod(int(m), NT)
                out[:, mi*TS:(mi+1)*TS, mj*TS:(mj+1)*TS] += \
                    accp.reshape(-1, TS, TS)
    return out.astype(np.float32)


# ======================================================================
# Device program
# ======================================================================
NROWS_GATHER = 23032          # max gather row index + 1 (per 180-angle band)
BAND_ELEMS = NCLS * T * B     # 2,949,120 elements per angle band
FILT_ELEMS = PHI * T * B      # 11,796,480
GROW = GWIN * B               # 1152 elements per gathered row
GI_P = (0, 2, 4, 6)           # sign +1 members (lhsT tap = k)
GI_M = (1, 3, 5, 7)           # sign -1 members (lhsT tap = KWIN-1-k)
GI_ORDER = GI_P + GI_M        # oslots slot -> gi

_nc_cache = {}

# device-program variants (A/B tunables)
CFG = dict(f2_m128=True, bp_m128=True, legacy_bp=True, wgen=False,
           f2_psum_store=False, repack_pool=False, pair_gather=False,
           gq=1, wb_predup=True, fused_store=True, kw35=True,
           no_f2_mm=False, no_f2_store=False, no_f2_ld=False,
           no_gather=False, no_g2copy=False, no_wdma=False,
           no_bp_mm=False, no_bp_store=False)

def _build_nc(phases=("filter", "backproj"), repeat=1, cfg=None):
    global _nc_cache
    cfg = dict(CFG if cfg is None else cfg)
    key = (tuple(phases), repeat, tuple(sorted(cfg.items())))
    if key in _nc_cache:
        return _nc_cache[key]
    phases = tuple(phases)
    import concourse.bass as bass
    import concourse.bacc as bacc
    import concourse.mybir as mybir
    import concourse.tile as tile

    plan = build_plan()
    signs = plan["signs"]; bands = plan["bands"]
    assert all(signs[gi] == 1 for gi in GI_P)
    assert all(signs[gi] == -1 for gi in GI_M)
    bf = mybir.dt.bfloat16
    f32 = mybir.dt.float32

    nc = bacc.Bacc(None, target_bir_lowering=False,
                   num_swdge_queues=max(1, int(cfg.get("gq", 1))))
    sinoT = nc.dram_tensor("sinoT", [128, 4, PHI * B], bf, kind="ExternalInput")
    kern = nc.dram_tensor("kern", [384, 1], f32, kind="ExternalInput")
    irm = nc.dram_tensor("irm", [384, 512], f32, kind="ExternalInput")
    coffs = nc.dram_tensor("coffs", [4, 128, 1], mybir.dt.int32, kind="ExternalInput")
    if cfg["wgen"]:
        wa = wb = None
        geo_d = nc.dram_tensor("geo", [3, NU_PER_CORE, 2, 128], f32,
                               kind="ExternalInput")
        pij_d = nc.dram_tensor("pij", [3, 256], f32, kind="ExternalInput")
    else:
        kwv = 35 if cfg["kw35"] else KWIN
        wa = nc.dram_tensor("wa", [NU_PER_CORE, 128, kwv, 256], bf, kind="ExternalInput")
        wb = nc.dram_tensor("wb", [NU_PER_CORE,
                                   128 if cfg["wb_predup"] else 52,
                                   kwv, 256], bf, kind="ExternalInput")
    gidx = nc.dram_tensor("gidx", [NU_PER_CORE, 4, 128, 24], mybir.dt.int16,
                          kind="ExternalInput")
    oslots = nc.dram_tensor("oslots", [NU_PER_CORE, NG, B, 256], bf,
                            kind="ExternalOutput")
    h2d = nc.dram_tensor("h2d", [1024, 1], f32)
    # filt split into per-band tensors: band-b gathers depend only on
    # band-b stores, so bands 0..2 gathers prefetch during late F2
    # (single-tensor filt serialized ALL gathers behind ALL of F2).
    filts = [nc.dram_tensor(f"filt{b}", [BAND_ELEMS + 256], bf)
             for b in range(4)]
    ftens = [f.tensor if hasattr(f, "tensor") else f for f in filts]

    def _emit_filter(tc, sfx=""):
        with (tc.tile_pool(name="fsb" + sfx, bufs=1) as fsb,
              tc.tile_pool(name="fst" + sfx, bufs=4) as fst,
              tc.tile_pool(name="fwork" + sfx, bufs=4) as fwork,
              tc.tile_pool(name="fps" + sfx, bufs=2, space="PSUM") as fps):
            # F0: h = IRm.T @ kern
            hps = fps.tile([1, 512], f32, tag="hps")
            for ch, (k0, ksz) in enumerate(((0, 128), (128, 128), (256, 128))):
                kt = fsb.tile([ksz, 1], f32, tag=f"kt{ch}")
                nc.sync.dma_start(kt[:], kern[k0:k0 + ksz, :])
                irt = fsb.tile([ksz, 512], f32, tag=f"irt{ch}")
                nc.sync.dma_start(irt[:], irm[k0:k0 + ksz, :])
                nc.tensor.matmul(hps[:], lhsT=kt[:], rhs=irt[:],
                                 start=(ch == 0), stop=(ch == 2))
            h2sb = fsb.tile([1, 1024], f32, tag="h2sb")
            nc.vector.tensor_copy(h2sb[:, 0:512], hps[:])
            nc.vector.tensor_copy(h2sb[:, 512:1024], hps[:])
            nc.sync.dma_start(h2d[:].rearrange("a b -> b a"), h2sb[:])

            # F1: circulant chunks C[ch] = h2[512 - s + t], bf16
            csb = []
            for ch in range(4):
                co = fsb.tile([128, 1], mybir.dt.int32, tag=f"co{ch}")
                nc.sync.dma_start(co[:], coffs[ch])
                cf = fsb.tile([128, 512], f32, tag=f"cf{ch}")
                nc.gpsimd.indirect_dma_start(
                    out=cf[:], out_offset=None, in_=h2d[:],
                    in_offset=bass.IndirectOffsetOnAxis(ap=co[:], axis=0))
                cb = fsb.tile([128, 512], bf, tag=f"cb{ch}")
                nc.vector.tensor_copy(cb[:], cf[:])
                csb.append(cb)

            # F2: filtered (transposed): psum[t, (phi,b)] = C[s,t].T @ sinoT
            # 1024-wide column blocks; one fused load and one fused store
            # per block to keep the DMA instruction count low.
            CSZ = 1024
            for cs in range(0, PHI * B, CSZ):
                csz = min(CSZ, PHI * B - cs)
                nphi = csz // B
                st = fst.tile([128, 4, CSZ], bf, tag="st")
                if cfg["no_f2_ld"]:
                    nc.vector.memset(st[:].rearrange("p c x -> p (c x)"), 0.0)
                else:
                    nc.sync.dma_start(st[:, :, :csz], sinoT[:, :, cs:cs + csz])
                fb = None
                if not cfg["f2_psum_store"]:
                    fb = fwork.tile([128, 4, CSZ], bf, tag="fb")
                for tch in range(4):
                    fp = fps.tile([128, CSZ], f32, tag="fp")
                    for h0 in range(0, csz, 512):
                        hsz = min(512, csz - h0)
                        if cfg["no_f2_mm"] and h0 == 0:
                            nc.vector.memset(fp[:, :csz], 0.0)
                        for ch in range(4):
                            if cfg["no_f2_mm"]:
                                continue
                            if cfg["f2_m128"]:
                                nc.tensor.matmul(
                                    fp[:, h0:h0 + hsz],
                                    lhsT=csb[ch][:, tch * 128:
                                                 tch * 128 + 128],
                                    rhs=st[:, ch, h0:h0 + hsz],
                                    start=(ch == 0), stop=(ch == 3),
                                    tile_position=(0, 0))
                                continue
                            for soff in (0, 64):
                                nc.tensor.matmul(
                                    fp[soff:soff + 64, h0:h0 + hsz],
                                    lhsT=csb[ch][:, tch * 128 + soff:
                                                 tch * 128 + soff + 64],
                                    rhs=st[:, ch, h0:h0 + hsz],
                                    start=(ch == 0), stop=(ch == 3),
                                    tile_position=(0, soff))
                    phi0 = cs // B
                    if cfg["f2_psum_store"]:
                        if not cfg["no_f2_store"]:
                            out_ap = bass.AP(ftens,
                                             phi0 * (T * B) + tch * 128 * B,
                                             [[B, 128], [T * B, nphi], [1, B]])
                            nc.gpsimd.dma_start(
                                out_ap,
                                fp[:, :csz].rearrange("p (f b) -> p f b", b=B))
                        continue
                    nc.vector.tensor_copy(fb[:, tch, :csz], fp[:, :csz])
                if cfg["f2_psum_store"]:
                    continue
                # store: partition p = t (tch*128+p); free = (phi_rel, b);
                # blocks crossing a 180-angle band boundary split in two
                phi0 = cs // B
                for tch in range(4):
                    if cfg["no_f2_store"]:
                        continue
                    p_lo = phi0
                    while p_lo < phi0 + nphi:
                        bnd = p_lo // 180
                        p_hi = min(phi0 + nphi, (bnd + 1) * 180)
                        n = p_hi - p_lo
                        off = (p_lo - bnd * 180) * (T * B) + tch * 128 * B
                        out_ap = bass.AP(ftens[bnd], off,
                                         [[B, 128], [T * B, n], [1, B]])
                        nc.sync.dma_start(
                            out_ap,
                            fb[:, tch, (p_lo - phi0) * B:(p_hi - phi0) * B]
                            .rearrange("p (f b) -> p f b", b=B))
                        p_lo = p_hi

    # ---------------- backprojection ----------------
    # Direct-gather layout: per nu one slot-major tile
    # G[128, 4 slots(=bands), 3 chunks, GWIN, B]; each band-gather writes the
    # contiguous G[:, band] slice (chunk0 = P-member rows, chunk1 = M-member
    # rows, chunk2 = chunk1 P/M pair). lhsT APs stride across slots, so no
    # DVE repack copies are needed. PSUM slot order = band order (host merge
    # uses plan["slot_gis"]).
    def _emit_backproj_direct(tc, sfx=""):
        with (tc.tile_pool(name="bsb" + sfx, bufs=1) as bsb,
              tc.tile_pool(name="bg" + sfx, bufs=3) as bg,
              tc.tile_pool(name="bw" + sfx, bufs=2) as bw,
              tc.tile_pool(name="bst" + sfx, bufs=2) as bst,
              tc.tile_pool(name="bps" + sfx, bufs=2, space="PSUM") as bps):
            idxsb = bsb.tile([128, NU_PER_CORE * 4 * 24], mybir.dt.int16, tag="idx")
            nc.sync.dma_start(
                idxsb[:].rearrange("p (n g w) -> p n g w", n=NU_PER_CORE, g=4),
                gidx[:].transpose([2, 0, 1, 3]))
            otens = oslots.tensor if hasattr(oslots, "tensor") else oslots
            for nu in range(NU_PER_CORE):
                G = bg.tile([128, 4, 3, GWIN, B], bf, tag="G")
                for band in range(4):
                    in_ap = bass.AP(ftens[band], 0,
                                    [[128, NROWS_GATHER], [1, GROW]])
                    if cfg["no_gather"]:
                        nc.vector.memset(
                            G[:, band].rearrange("p c k b -> p (c k b)"), 0.0)
                        continue
                    nc.gpsimd.dma_gather(
                        out_ap=G[:, band].rearrange("p c k b -> p c (k b)"),
                        in_ap=in_ap,
                        idxs_ap=idxsb[:, (nu * 4 + band) * 24:
                                      (nu * 4 + band + 1) * 24],
                        num_idxs=384, num_idxs_reg=384,
                        elem_size=GROW, elem_step=128,
                        single_packet=False)
                wat = bw.tile([128, KWIN, 256], bf, tag="wa")
                if cfg["no_wdma"]:
                    nc.vector.memset(wat[:].rearrange("p k x -> p (k x)"), 0.0)
                else:
                    nc.sync.dma_start(wat[:], wa[nu])
                wbt = bw.tile([128, KWIN, 256], bf, tag="wb")
                if cfg["no_wdma"]:
                    nc.vector.memset(wbt[:].rearrange("p k x -> p (k x)"), 0.0)
                else:
                    nc.sync.dma_start(wbt[0:52], wb[nu])
                    nc.sync.dma_start(wbt[64:116], wbt[0:52])
                psP = bps.tile([128, 256], f32, tag="psP")
                psM = bps.tile([128, 256], f32, tag="psM")
                if cfg["no_bp_mm"]:
                    nc.vector.memset(psP[:], 0.0)
                    nc.vector.memset(psM[:], 0.0)
                for cchunk in range(0 if cfg["no_bp_mm"] else 2):
                    for k in range(KWIN):
                        km = KWIN - 1 - k
                        for ps, creg, half, ks in ((psP, 0, 0, k),
                                                   (psM, 1, 1, km)):
                            if cfg["bp_m128"]:
                                if cchunk == 0:
                                    lhs = G[:, :, creg, ks, :]
                                    rhs = wat[:, k, :]
                                    tp = (0, 0)
                                elif half == 0:
                                    lhs = G[0:52, :, 2, k, :]
                                    rhs = wbt[0:52, k, :]
                                    tp = (0, 0)
                                else:
                                    lhs = G[64:116, :, 2, km, :]
                                    rhs = wbt[64:116, k, :]
                                    tp = (64, 0)
                                nc.tensor.matmul(
                                    ps[:, :],
                                    lhsT=lhs, rhs=rhs,
                                    start=(cchunk == 0 and k == 0),
                                    stop=(cchunk == 1 and k == kwv - 1),
                                    tile_position=tp)
                                continue
                            for p in range(2):
                                if cchunk == 0:
                                    lhs = G[:, 2 * p:2 * p + 2, creg, ks, :]
                                    rhs = wat[:, k, :]
                                    tp = (0, p * 64)
                                elif half == 0:
                                    lhs = G[0:52, 2 * p:2 * p + 2, 2, k, :]
                                    rhs = wbt[0:52, k, :]
                                    tp = (0, p * 64)
                                else:
                                    lhs = G[64:116, 2 * p:2 * p + 2, 2, km, :]
                                    rhs = wbt[64:116, k, :]
                                    tp = (64, p * 64)
                                nc.tensor.matmul(
                                    ps[p * 64:(p + 1) * 64, :],
                                    lhsT=lhs, rhs=rhs,
                                    start=(cchunk == 0 and k == 0),
                                    stop=(cchunk == 1 and k == kwv - 1),
                                    tile_position=tp)
                for half, ps in (() if cfg["no_bp_store"] else
                                 ((0, psP), (1, psM))):
                    stg = bst.tile([128, 256], bf, tag="stg")
                    nc.vector.tensor_copy(stg[:], ps[:])
                    out_ap = bass.AP(otens,
                                     nu * NG * B * 256 + half * 4 * B * 256,
                                     [[256, 128], [1, 256]])
                    nc.sync.dma_start(out_ap, stg[:])

    def _emit_backproj(tc, sfx=""):
        f16 = mybir.dt.float16
        with (tc.tile_pool(name="bsb" + sfx, bufs=1) as bsb,
              tc.tile_pool(name="bg" + sfx,
                           bufs=1 if cfg["pair_gather"] else 2) as bg,
              tc.tile_pool(name="bg2" + sfx, bufs=2) as bg2,
              tc.tile_pool(name="bw" + sfx, bufs=2) as bw,
              tc.tile_pool(name="bst" + sfx, bufs=1) as bst,
              tc.tile_pool(name="bgeo" + sfx, bufs=1) as bgeo,
              tc.tile_pool(name="bau" + sfx, bufs=2) as bau,
              tc.tile_pool(name="ba" + sfx, bufs=4) as ba,
              tc.tile_pool(name="bu" + sfx, bufs=2, space="PSUM") as bu,
              tc.tile_pool(name="bps" + sfx, bufs=2, space="PSUM") as bps):
            if cfg["wgen"]:
                pijt = bsb.tile([3, 256], f32, tag="pij")
                nc.sync.dma_start(pijt[:], pij_d[:])
                kneg = bsb.tile([128, KWIN], f32, tag="kneg")
                for k in range(KWIN):
                    nc.gpsimd.memset(kneg[:, k:k + 1], -float(k))
            idxsb = bsb.tile([128, NU_PER_CORE * 4 * 24], mybir.dt.int16, tag="idx")
            nc.sync.dma_start(
                idxsb[:].rearrange("p (n g w) -> p n g w", n=NU_PER_CORE, g=4),
                gidx[:].transpose([2, 0, 1, 3]))
            otens = oslots.tensor if hasattr(oslots, "tensor") else oslots
            for nu in range(NU_PER_CORE):
                # one fused gather per (nu, band): free-slot 0 = P-member
                # chunk0, 1 = M-member chunk0, 2 = chunk1 pair (P classes on
                # partitions 0..51, M on 64..115, dummy idx-0 rows elsewhere)
                gts = []
                for band in range(4):
                    gt = bg.tile([128, 3, GWIN, B], bf, tag=f"gt{band}")
                    in_ap = bass.AP(ftens[band], 0,
                                    [[128, NROWS_GATHER], [1, GROW]])
                    if cfg["no_gather"]:
                        nc.vector.memset(
                            gt[:].rearrange("p c k b -> p (c k b)"), 0.0)
                        gts.append(gt)
                        continue
                    nc.gpsimd.dma_gather(
                        out_ap=gt[:].rearrange("p c k b -> p c (k b)"),
                        in_ap=in_ap,
                        idxs_ap=idxsb[:, (nu * 4 + band) * 24:
                                      (nu * 4 + band + 1) * 24],
                        num_idxs=384, num_idxs_reg=384,
                        elem_size=GROW, elem_step=128,
                        single_packet=False)
                    gts.append(gt)
                # rearrange band tiles into pair-contiguous layout so
                # same-half slot pairs form 64-wide lhsT slices (64-col
                # LDWEIGHTS measured at +3ns/MM vs 32-col; halves MM count)
                g2P = bg2.tile([128, 2, GWIN, 4, B], bf, tag="g2P")
                g2M = bg2.tile([128, 2, GWIN, 4, B], bf, tag="g2M")
                if cfg["no_g2copy"]:
                    for g2 in (g2P, g2M):
                        nc.vector.memset(
                            g2[:].rearrange("p c k s b -> p (c k s b)"), 0.0)
                ceng = nc.gpsimd if cfg["repack_pool"] else nc.vector
                for g2, gis, fslot in (() if cfg["no_g2copy"] else
                                       ((g2P, GI_P, 0), (g2M, GI_M, 1))):
                    for s in range(4):
                        gt = gts[int(bands[gis[s]])]
                        ceng.tensor_copy(g2[:, 0, :, s, :], gt[:, fslot])
                        if fslot == 0:
                            ceng.tensor_copy(g2[0:52, 1, :, s, :],
                                             gt[0:52, 2])
                        else:
                            ceng.tensor_copy(g2[64:116, 1, :, s, :],
                                             gt[64:116, 2])
                wat = bw.tile([128, kwv, 256], bf, tag="wa")
                wbt = bw.tile([128, kwv, 256], bf, tag="wb")
                if cfg["wgen"]:
                    # on-device W tables: u = A*pi + B*pj + C2 via K=3 matmul,
                    # then per tap  -W[k] = min(|u-k|, 1) - 1  (sign restored
                    # in the psum->SBUF output copy).
                    geot = bgeo.tile([3, 2, 128], f32, tag="geo")
                    nc.sync.dma_start(geot[:], geo_d[:, nu])
                    for c2, wt in ((0, wat), (1, wbt)):
                        ups = bu.tile([128, 256], f32, tag="u")
                        nc.tensor.matmul(ups[:], lhsT=geot[:, c2, :],
                                         rhs=pijt[:], start=True, stop=True)
                        usb = bau.tile([128, 256], f32, tag="usb")
                        nc.vector.tensor_copy(usb[:], ups[:])
                        for k in range(kwv):
                            ak = ba.tile([128, 256], f16, tag="ak")
                            nc.scalar.activation(
                                ak[:], usb[:],
                                mybir.ActivationFunctionType.Abs,
                                bias=kneg[:, k:k + 1], scale=1.0)
                            eng2 = nc.gpsimd if (k + c2) % 2 == 0 else nc.vector
                            eng2.tensor_scalar(
                                wt[:, k, :], ak[:], 1.0, 1.0,
                                mybir.AluOpType.min,
                                mybir.AluOpType.subtract)
                elif cfg["no_wdma"]:
                    nc.vector.memset(wat[:].rearrange("p k x -> p (k x)"), 0.0)
                    nc.vector.memset(wbt[:].rearrange("p k x -> p (k x)"), 0.0)
                elif cfg["wb_predup"]:
                    nc.sync.dma_start(wat[:], wa[nu])
                    # chunk-B weights shipped pre-duplicated at both
                    # partition bases: one load, no serial SBUF->SBUF dup
                    nc.sync.dma_start(wbt[:], wb[nu])
                else:
                    nc.sync.dma_start(wat[:], wa[nu])
                    # chunk-B weights at both partition bases used by the
                    # pair; second copy moves SBUF->SBUF to spare HBM
                    nc.sync.dma_start(wbt[0:52], wb[nu])
                    nc.sync.dma_start(wbt[64:116], wbt[0:52])
                psP = bps.tile([128, 256], f32, tag="psP")
                psM = bps.tile([128, 256], f32, tag="psM")
                if cfg["no_bp_mm"]:
                    nc.vector.memset(psP[:], 0.0)
                    nc.vector.memset(psM[:], 0.0)
                for cchunk in range(0 if cfg["no_bp_mm"] else 2):
                    for k in range(kwv):
                        km = GWIN - 1 - k
                        for ps, g2, half, ks in ((psP, g2P, 0, k),
                                                 (psM, g2M, 1, km)):
                            if cfg["bp_m128"]:
                                if cchunk == 0:
                                    lhs = g2[:, 0, ks, :, :]
                                    rhs = wat[:, k, :]
                                    tp = (0, 0)
                                elif half == 0:
                                    lhs = g2[0:52, 1, ks, :, :]
                                    rhs = wbt[0:52, k, :]
                                    tp = (0, 0)
                                else:
                                    lhs = g2[64:116, 1, ks, :, :]
                                    rhs = wbt[64:116, k, :]
                                    tp = (64, 0)
                                nc.tensor.matmul(
                                    ps[:, :],
                                    lhsT=lhs, rhs=rhs,
                                    start=(cchunk == 0 and k == 0),
                                    stop=(cchunk == 1 and k == kwv - 1),
                                    tile_position=tp)
                                continue
                            for p in range(2):
                                if cchunk == 0:
                                    lhs = g2[:, 0, ks, 2 * p:2 * p + 2, :]
                                    rhs = wat[:, k, :]
                                    tp = (0, p * 64)
                                elif half == 0:
                                    lhs = g2[0:52, 1, ks, 2 * p:2 * p + 2, :]
                                    rhs = wbt[0:52, k, :]
                                    tp = (0, p * 64)
                                else:
                                    lhs = g2[64:116, 1, ks, 2 * p:2 * p + 2, :]
                                    rhs = wbt[64:116, k, :]
                                    tp = (64, p * 64)
                                nc.tensor.matmul(
                                    ps[p * 64:(p + 1) * 64, :],
                                    lhsT=lhs, rhs=rhs,
                                    start=(cchunk == 0 and k == 0),
                                    stop=(cchunk == 1 and k == kwv - 1),
                                    tile_position=tp)
                if cfg["fused_store"] and not cfg["no_bp_store"]:
                    stg = bst.tile([128, 2, 256], bf, tag="stg")
                    for half, ps in ((0, psP), (1, psM)):
                        if cfg["wgen"]:
                            nc.vector.tensor_scalar(stg[:, half, :], ps[:],
                                                    -1.0, None,
                                                    mybir.AluOpType.mult)
                        else:
                            nc.vector.tensor_copy(stg[:, half, :], ps[:])
                    out_ap = bass.AP(otens, nu * NG * B * 256,
                                     [[256, 128], [4 * B * 256, 2], [1, 256]])
                    nc.sync.dma_start(out_ap, stg[:])
                else:
                  for half, ps in (() if cfg["no_bp_store"] else
                                   ((0, psP), (1, psM))):
                    stg = bst.tile([128, 256], bf, tag="stg")
                    if cfg["wgen"]:
                        nc.vector.tensor_scalar(stg[:], ps[:], -1.0, None,
                                                mybir.AluOpType.mult)
                    else:
                        nc.vector.tensor_copy(stg[:], ps[:])
                    out_ap = bass.AP(otens,
                                     nu * NG * B * 256 + half * 4 * B * 256,
                                     [[256, 128], [1, 256]])
                    nc.sync.dma_start(out_ap, stg[:])

    with tile.TileContext(nc) as tc:
        for r in range(repeat):
            sfx = "" if repeat == 1 else f"r{r}"
            if "filter" in phases:
                _emit_filter(tc, sfx)
            if "backproj" in phases:
                if cfg["legacy_bp"]:
                    _emit_backproj(tc, sfx)
                else:
                    _emit_backproj_direct(tc, sfx)
    nc.compile()
    _nc_cache[key] = nc
    return nc


def _prep_inputs(sinos, kern_in, cfg=None):
    cfg = dict(CFG if cfg is None else cfg)
    plan = build_plan()
    bf = ml_dtypes.bfloat16
    kern_t = np.zeros((384, 1), np.float32)
    kern_t[:257, 0] = np.asarray(kern_in, np.float32)
    irm_pad = np.zeros((384, 512), np.float32)
    irm_pad[:257] = plan["IRm"]
    if cfg["wgen"]:
        # W tables are generated on device unscaled; fold DPHI into the
        # filter so filt carries it instead.
        irm_pad *= np.float32(DPHI)
        pij = np.zeros((3, 256), np.float32)
        pij[0] = np.arange(256) // 16
        pij[1] = np.arange(256) % 16
        pij[2] = 1.0
    # full sinogram, transposed to [t, (phi, b)], replicated on every core;
    # layout [t%128, t//128, (phi, b)] so each filter block loads in one DMA
    st = np.asarray(sinos, np.float64).transpose(2, 1, 0).reshape(T, PHI * B)
    st = st.reshape(4, 128, PHI * B).transpose(1, 0, 2).astype(bf)
    in_maps = []
    for c in range(NCORE):
        m = {
            "sinoT": st,
            "kern": kern_t,
            "irm": irm_pad,
            "coffs": plan["coffs"],
            "gidx": plan["GIs"][c].reshape(NU_PER_CORE, 4, 128, 24),
        }
        if cfg["wgen"]:
            m["geo"] = _build_geo(plan["core_nus"][c])
            m["pij"] = pij
        else:
            kwv = 35 if cfg["kw35"] else KWIN
            m["wa"] = plan["WAs"][c][:, :, :kwv]
            if cfg["wb_predup"]:
                wbp = np.zeros((NU_PER_CORE, 128, kwv, 256),
                               plan["WBs"][c].dtype)
                wbp[:, 0:52] = plan["WBs"][c][:, :, :kwv]
                wbp[:, 64:116] = plan["WBs"][c][:, :, :kwv]
                m["wb"] = wbp
            else:
                m["wb"] = plan["WBs"][c][:, :, :kwv]
        if cfg["pair_gather"]:
            g = plan["GIs"][c].reshape(NU_PER_CORE, 4, 128, 24)
            m["gidx"] = np.concatenate(
                [g[0::2], g[1::2]], axis=-1)  # [8, 4, 128, 48]
        in_maps.append(m)
    return in_maps


def _merge_outputs(results, slot_order=None):
    plan = build_plan()
    if slot_order is None:
        slot_order = GI_ORDER if CFG["legacy_bp"] else plan["slot_gis"]
    out = np.zeros((B, H, W), np.float64)
    for c in range(NCORE):
        slots = results[c]["oslots"].astype(np.float64)   # [16, 8, 32, 256]
        for j in range(NU_PER_CORE):
            for idx, gi in enumerate(slot_order):
                m = int(plan["merges"][c][j, gi, 0])
                pm = plan["gP"][gi]
                accp = np.zeros((B, TS * TS))
                accp[:, pm] = slots[j, idx]
                mi, mj = divmod(m, NT)
                out[:, mi * TS:(mi + 1) * TS, mj * TS:(mj + 1) * TS] += \
                    accp.reshape(B, TS, TS)
    return out.astype(np.float32)


def kernel(sinos, kernel):
    from concourse.bass_utils import run_bass_kernel_spmd
    sinos = np.asarray(sinos)
    kern_in = np.asarray(kernel)
    nc = _build_nc()
    in_maps = _prep_inputs(sinos, kern_in)
    res = run_bass_kernel_spmd(nc, in_maps, list(range(NCORE)))
    return _merge_outputs(res.results)

